# revision 11
# baseline (speedup 1.0000x reference)
"""Distributed GCN (5x GraphConv(add) + residual/ReLU + mean-pool + linear)
for 8 Trainium2 NeuronCores.

Sharding: nodes partitioned contiguously across cores (1280 nodes/core, padded
to 10240). Each core owns the edges whose *destination* lands in its shard.
Aggregation is computed as A@(x@Wr): project first (p = x@Wr), AllGather p,
gather p[src[e]] rows with SWDGE dma_gather, then reduce edge tiles onto
destination nodes with one-hot segment matmuls on the tensor engine.

fp8 pipeline: p is stored/gathered as fp8e4m3 (halves gather DMA bytes), and
all heavy matmuls run in fp8 DoubleRow perf mode (2 K-slabs per instruction,
0.5 cycles/row): segment one-hot pairs, Ws root pairs, and Wr projection as an
fp8 hi + 2^7-scaled-lo pair (the lo PSUM is recombined at 2^-7 so the
effective Wr precision exceeds bf16). Per-layer power-of-2 x-scales (computed
from a host forward pass) keep activations inside fp8e4m3 range; they are
folded into the host-prepped weights so no extra device ops are needed except
a scaled copy when building the transposed x. The residual stream, pooling,
and final linear stay bf16/fp32.
"""

import numpy as np
import ml_dtypes

BF16 = ml_dtypes.bfloat16
FP8 = ml_dtypes.float8_e4m3      # TRN float8e4 (max normal 240)

N, E, D, OUT, G = 10000, 160000, 512, 128, 64
NCORES, P = 8, 128
NBLK = 10                     # 128-node blocks per core
NC_NODES = NBLK * P           # 1280
NPAD = NCORES * NC_NODES      # 10240
NLAYERS = 5
KD = D // P                   # 4 chunks of in-channels
LO_SHIFT = 7                  # Wr lo-residual stored scaled by 2^LO_SHIFT


def _wrap_idx(a):
    """[L] ints -> [128, L//16] int16 SWDGE index layout (16-partition wrap,
    replicated for the 8 Q7 cores)."""
    L = len(a)
    w = a.astype(np.int16).reshape(L // 16, 16).T
    return np.ascontiguousarray(np.tile(w, (8, 1)))


def _forward_stats(x, src, dst, batch, inputs):
    """fp32 host forward pass -> per-layer max|x_in| and max|p|."""
    try:
        import scipy.sparse as sp
        A = sp.csr_matrix((np.ones(len(src), np.float32), (dst, src)), shape=(N, N))
        segsum = lambda v: A @ v
    except Exception:
        def segsum(v):
            out = np.zeros((N, v.shape[1]), np.float32)
            np.add.at(out, dst, v[src])
            return out
    xmax, pmax = [], []
    xx = x
    for l in range(NLAYERS):
        Wr = np.asarray(inputs[f"Wr{l+1}"], np.float32)
        Ws = np.asarray(inputs[f"Ws{l+1}"], np.float32)
        b = np.asarray(inputs[f"b{l+1}"], np.float32)
        xmax.append(float(np.abs(xx).max()))
        p = xx @ Wr
        pmax.append(float(np.abs(p).max()))
        val = segsum(p) + xx @ Ws + b + xx
        if l < NLAYERS - 1:
            val = np.maximum(val, 0)
        xx = val
    return xmax, pmax


def _prep(inputs):
    x = np.asarray(inputs["x"], np.float32)
    ei = np.asarray(inputs["edge_index"]).astype(np.int64)
    batch = np.asarray(inputs["batch"]).astype(np.int64)
    src, dst = ei[0], ei[1]

    xmax, pmax = _forward_stats(x, src, dst, batch, inputs)
    # x scale: keep x * 2^-S[l] <= ~200 (fp8e4m3 max normal 240)
    S = [max(0, int(np.ceil(np.log2(m / 200.0)))) if m > 200 else 0 for m in xmax]
    # p scale: keep p * 2^-t <= ~200 (one-hot value 2^t restores it)
    pm = max(pmax[l] * 2.0 ** 0 for l in range(NLAYERS))
    t_sh = max(0, int(np.ceil(np.log2(pm / 200.0)))) if pm > 200 else 0

    order = np.argsort(dst, kind="stable")
    ds_, ss_ = dst[order], src[order]
    starts = np.searchsorted(ds_, np.arange(0, NPAD + 1, P))
    counts = np.diff(starts)
    T_pad = max(1, int(np.ceil(counts.max() / P)))
    L = T_pad * P

    xp = np.zeros((NPAD, D), np.float32)
    xp[:N] = x

    counts_g = np.bincount(batch, minlength=G)[:G]
    inv = (1.0 / np.maximum(counts_g, 1.0)).astype(np.float32)

    oh_val = float(2.0 ** t_sh)
    per_core = []
    for c in range(NCORES):
        idx_blocks = []
        oh_flat = np.zeros((P, NBLK * T_pad * P), FP8)
        goh = np.zeros((P, NBLK * G), BF16)
        for b in range(NBLK):
            gb = c * NBLK + b
            lo = gb * P
            s0, s1 = int(starts[gb]), int(starts[gb + 1])
            n = s1 - s0
            srcs = np.zeros(L, np.int64)
            srcs[:n] = ss_[s0:s1]
            dloc = ds_[s0:s1] - lo
            oh = np.zeros((L, P), FP8)
            oh[np.arange(n), dloc] = oh_val
            idx_blocks.append(_wrap_idx(srcs))
            oh_flat[:, b * T_pad * P:(b + 1) * T_pad * P] = (
                oh.reshape(T_pad, P, P).transpose(1, 0, 2).reshape(P, T_pad * P))
            nodes = lo + np.arange(P)
            valid = nodes < N
            goh[valid, b * G + batch[nodes[valid]]] = 1

        shard = xp[c * NC_NODES:(c + 1) * NC_NODES].astype(BF16)
        xt0 = np.ascontiguousarray(
            (shard.astype(np.float32) * 2.0 ** -S[0]).astype(FP8)
            .T.reshape(KD, P, NC_NODES).transpose(1, 0, 2))
        per_core.append(dict(
            x_shard=np.ascontiguousarray(shard),
            xt0=xt0,
            ohot=oh_flat,
            idxe=np.ascontiguousarray(np.concatenate(idx_blocks, axis=1)),
            goh=goh,
        ))

    wr_hi = np.zeros((P, NLAYERS, KD, D), FP8)
    wr_lo = np.zeros((P, NLAYERS, KD, D), FP8)
    ws_q = np.zeros((P, NLAYERS, KD, D), FP8)
    bias = np.zeros((P, NLAYERS, D), BF16)
    bias_zero = True
    for l in range(NLAYERS):
        WR = np.asarray(inputs[f"Wr{l+1}"], np.float32) * 2.0 ** (S[l] - t_sh)
        hi = WR.astype(FP8)
        lo = ((WR - hi.astype(np.float32)) * 2.0 ** LO_SHIFT).astype(FP8)
        WS = (np.asarray(inputs[f"Ws{l+1}"], np.float32) * 2.0 ** S[l]).astype(FP8)
        for arr, dst_arr in ((hi, wr_hi), (lo, wr_lo), (WS, ws_q)):
            dst_arr[:, l] = arr.reshape(KD, P, D).transpose(1, 0, 2)
        b_l = np.asarray(inputs[f"b{l+1}"], np.float32)
        if np.any(b_l != 0):
            bias_zero = False
        bias[0, l] = b_l.astype(BF16)
    ones_e0 = np.zeros((P, P), BF16)
    ones_e0[0, :] = 1
    wlin = np.ascontiguousarray(
        np.asarray(inputs["Wlin"], np.float32).reshape(KD, P, OUT)
        .transpose(1, 0, 2).astype(BF16))
    blin = np.asarray(inputs["blin"], np.float32).reshape(OUT, 1).astype(np.float32)
    shared = dict(
        wr_hi=wr_hi, wr_lo=wr_lo, ws=ws_q, bias=bias, ones=ones_e0,
        wlin=wlin, blin=blin,
        invt=np.ascontiguousarray(np.tile(inv, (P, KD)).astype(np.float32)),
        ident=np.eye(P, dtype=BF16),
    )
    meta = dict(T_pad=T_pad, S=S, t_sh=t_sh, bias_zero=bias_zero)
    return per_core, shared, meta


def _unwrap(w, L):
    """inverse of _wrap_idx: [128, L//16] -> [L]"""
    return np.ascontiguousarray(w[:16].T).reshape(-1)[:L].astype(np.int64)


def emulate(inputs):
    """Numpy emulation of the exact device dataflow (fp8/bf16 casts included).
    Validates the host-side scale/index/one-hot bookkeeping."""
    per_core, shared, meta = _prep(inputs)
    T_pad, S, t_sh = meta["T_pad"], meta["S"], meta["t_sh"]
    L = T_pad * P
    f32 = np.float32

    xs = [pc["x_shard"].astype(f32) for pc in per_core]       # [1280, 512]
    xts = [pc["xt0"] for pc in per_core]                      # fp8 [P, KD, NC]
    for l in range(NLAYERS):
        wr_hi = np.concatenate([shared["wr_hi"][:, l, k, :] for k in range(KD)],
                               axis=0).astype(f32)
        wr_lo = np.concatenate([shared["wr_lo"][:, l, k, :] for k in range(KD)],
                               axis=0).astype(f32)
        ws_l = np.concatenate([shared["ws"][:, l, k, :] for k in range(KD)],
                              axis=0).astype(f32)
        b_l = shared["bias"][0, l].astype(f32)
        p_parts = []
        for c in range(NCORES):
            xm = np.concatenate(
                [xts[c][:, k, :].astype(f32) for k in range(KD)], axis=0).T
            p = xm @ wr_hi + (xm @ wr_lo) * 2.0 ** -LO_SHIFT
            p_parts.append(p.astype(FP8).astype(f32))
        p_full = np.concatenate(p_parts, axis=0)              # [10240, 512]
        new_xs, new_xts = [], []
        for c in range(NCORES):
            nx = np.zeros((NC_NODES, D), f32)
            nxt = np.zeros((P, KD, NC_NODES), FP8)
            xm_all = np.concatenate(
                [xts[c][:, k, :].astype(f32) for k in range(KD)], axis=0).T
            for b in range(NBLK):
                idx = _unwrap(
                    per_core[c]["idxe"][:, b * (L // 16):(b + 1) * (L // 16)], L)
                gath = p_full[idx]                             # [L, 512]
                acc = np.zeros((P, D), f32)
                for t in range(T_pad):
                    oh = per_core[c]["ohot"][
                        :, (b * T_pad + t) * P:(b * T_pad + t + 1) * P
                    ].astype(f32)                             # [128e, 128d]
                    acc += oh.T @ gath[t * P:(t + 1) * P]
                acc += xm_all[b * P:(b + 1) * P] @ ws_l + b_l
                val = acc + xs[c][b * P:(b + 1) * P]
                if l < NLAYERS - 1:
                    val = np.maximum(val, 0)
                val = val.astype(BF16).astype(f32)
                nx[b * P:(b + 1) * P] = val
                if l < NLAYERS - 1:
                    sc = 2.0 ** -S[l + 1]
                    nxt[:, :, b * P:(b + 1) * P] = (
                        (val * sc).astype(FP8).T.reshape(KD, P, P)
                        .transpose(1, 0, 2))
            new_xs.append(nx)
            new_xts.append(nxt)
        xs, xts = new_xs, new_xts
    # pooling
    pooled_T = np.zeros((D, G), f32)
    for c in range(NCORES):
        goh = per_core[c]["goh"].astype(f32)
        for b in range(NBLK):
            blk = xs[c][b * P:(b + 1) * P].astype(BF16).astype(f32)
            for j in range(KD):
                pooled_T[j * P:(j + 1) * P] += (
                    blk[:, j * P:(j + 1) * P].T @ goh[:, b * G:(b + 1) * G])
    inv = shared["invt"][0, :G].astype(f32)
    pooled_T = (pooled_T * inv[None, :]).astype(BF16).astype(f32)
    wlin = np.concatenate([shared["wlin"][:, k, :] for k in range(KD)],
                          axis=0).astype(f32)                 # [512, 128]
    out_T = wlin.T @ pooled_T + shared["blin"][:, :1]         # [128, 64]
    return np.ascontiguousarray(out_T.T).astype(np.float32)


def _build(meta, enable_asserts=False):
    import os
    T_pad = meta["T_pad"]
    S, bias_zero = meta["S"], meta["bias_zero"]
    n_layers = int(os.environ.get("GCN_LAYERS", NLAYERS))
    no_gather = bool(int(os.environ.get("GCN_NO_GATHER", "0")))
    no_cc = bool(int(os.environ.get("GCN_NO_CC", "0")))
    bP, bA, bT = (int(v) for v in os.environ.get("GCN_BANKS", "2,2,2").split(","))
    gbufs = int(os.environ.get("GCN_GBUFS", "4"))
    gsplit = int(os.environ.get("GCN_GSPLIT", "2"))
    # SWDGE ring: big enough for 2 block-gathers so descriptor generation for
    # block b+1 overlaps block b's DMA drain; 2 queues decouple them further
    scratch = int(os.environ.get("GCN_SCRATCH", "16384"))
    nqueues = int(os.environ.get("GCN_NQUEUES", "2"))
    import concourse.bass as bass
    import concourse.mybir as mybir
    import concourse.tile as tile
    from concourse import bacc

    F32 = mybir.dt.float32
    BF = mybir.dt.bfloat16
    F8 = mybir.dt.float8e4
    I16 = mybir.dt.int16
    ADD = mybir.AluOpType.add
    MUL = mybir.AluOpType.mult
    DR = mybir.MatmulPerfMode.DoubleRow
    L = T_pad * P
    RG = [list(range(NCORES))]
    NPAIR, TAIL = T_pad // 2, T_pad % 2

    nc = bacc.Bacc("TRN2", target_bir_lowering=False, debug=False,
                   enable_asserts=enable_asserts, num_devices=NCORES,
                   dynamic_dma_scratch_size=scratch,
                   num_swdge_queues=nqueues)

    # per-core inputs
    x_d = nc.dram_tensor("x_shard", [NC_NODES, D], BF, kind="ExternalInput")
    xt0_d = nc.dram_tensor("xt0", [P, KD, NC_NODES], F8, kind="ExternalInput")
    oh_d = nc.dram_tensor("ohot", [P, NBLK * T_pad * P], F8, kind="ExternalInput")
    idxe_d = nc.dram_tensor("idxe", [P, NBLK * (L // 16)], I16, kind="ExternalInput")
    goh_d = nc.dram_tensor("goh", [P, NBLK * G], BF, kind="ExternalInput")
    # shared inputs
    wrh_d = nc.dram_tensor("wr_hi", [P, NLAYERS, KD, D], F8, kind="ExternalInput")
    wrl_d = nc.dram_tensor("wr_lo", [P, NLAYERS, KD, D], F8, kind="ExternalInput")
    ws_d = nc.dram_tensor("ws", [P, NLAYERS, KD, D], F8, kind="ExternalInput")
    bias_d = nc.dram_tensor("bias", [P, NLAYERS, D], BF, kind="ExternalInput")
    ones_d = nc.dram_tensor("ones", [P, P], BF, kind="ExternalInput")
    wlin_d = nc.dram_tensor("wlin", [P, KD, OUT], BF, kind="ExternalInput")
    blin_d = nc.dram_tensor("blin", [OUT, 1], F32, kind="ExternalInput")
    invt_d = nc.dram_tensor("invt", [P, KD * G], F32, kind="ExternalInput")
    ident_d = nc.dram_tensor("ident", [P, P], BF, kind="ExternalInput")
    # internal DRAM (double-buffered by layer parity so the AllGather for
    # layer l+1 never WAR-depends on layer l's gathers)
    p_shard = [nc.dram_tensor(f"p_shard{i}", [NC_NODES, D], F8) for i in (0, 1)]
    p_full = [nc.dram_tensor(f"p_full{i}", [NPAD, D], F8, addr_space="Shared")
              for i in (0, 1)]
    pool_in = nc.dram_tensor("pool_in", [P, KD * G], F32)
    pool_out = nc.dram_tensor("pool_out", [P, KD * G], F32, addr_space="Shared")
    # output
    out_d = nc.dram_tensor("out_t", [OUT, G], F32, kind="ExternalOutput")

    with tile.TileContext(nc) as tc:
        with (
            tc.tile_pool(name="const", bufs=1) as const,
            tc.tile_pool(name="xs", bufs=2) as xpool,
            tc.tile_pool(name="xt", bufs=2) as xtpool,
            tc.tile_pool(name="gath", bufs=gbufs) as gpool,
            tc.tile_pool(name="small", bufs=int(os.environ.get("GCN_SBUFS", "4"))) as spool,
            tc.tile_pool(name="psP", bufs=bP, space="PSUM") as psP,
            tc.tile_pool(name="psA", bufs=bA, space="PSUM") as psA,
            tc.tile_pool(name="psS", bufs=1, space="PSUM") as psS,
            tc.tile_pool(name="psT", bufs=bT, space="PSUM") as psT,
        ):
            # ---- constants to SBUF
            oh_sb = const.tile([P, NBLK * T_pad * P], F8, tag="oh")
            nc.sync.dma_start(oh_sb[:], oh_d[:])
            idxe_sb = const.tile([P, NBLK * (L // 16)], I16, tag="idxe")
            nc.sync.dma_start(idxe_sb[:], idxe_d[:])
            ident_sb = const.tile([P, P], BF, tag="ident")
            nc.sync.dma_start(ident_sb[:], ident_d[:])
            goh_sb = const.tile([P, NBLK * G], BF, tag="goh")
            nc.sync.dma_start(goh_sb[:], goh_d[:])
            wrh_sb = const.tile([P, NLAYERS, KD, D], F8, tag="wrh")
            nc.sync.dma_start(wrh_sb[:], wrh_d[:])
            wrl_sb = const.tile([P, NLAYERS, KD, D], F8, tag="wrl")
            nc.sync.dma_start(wrl_sb[:], wrl_d[:])
            ws_sb = const.tile([P, NLAYERS, KD, D], F8, tag="ws")
            nc.sync.dma_start(ws_sb[:], ws_d[:])
            if not bias_zero:
                bias_sb = const.tile([P, NLAYERS, D], BF, tag="bias")
                nc.sync.dma_start(bias_sb[:], bias_d[:])
                ones_sb = const.tile([P, P], BF, tag="ones")
                nc.sync.dma_start(ones_sb[:], ones_d[:])
            wlin_sb = const.tile([P, KD, OUT], BF, tag="wlin")
            nc.sync.dma_start(wlin_sb[:], wlin_d[:])
            blin_sb = const.tile([OUT, 1], F32, tag="blin")
            nc.sync.dma_start(blin_sb[:], blin_d[:])
            invt_sb = const.tile([P, KD * G], F32, tag="invt")
            nc.sync.dma_start(invt_sb[:], invt_d[:])

            xs_cur = xpool.tile([P, NBLK, D], BF, tag="xs")
            nc.sync.dma_start(xs_cur[:], x_d.ap().rearrange("(b p) d -> p b d", p=P))
            xt_cur = xtpool.tile([P, KD, NC_NODES], F8, tag="xt")
            nc.sync.dma_start(xt_cur[:], xt0_d[:])

            def oh_pair(b, t):
                return oh_sb[:].rearrange("p (n q) -> p n q", q=P)[
                    :, b * T_pad + t:b * T_pad + t + 2, :]

            def emit_p_block(xt_src, layer, m, pbuf):
                """p[l=layer] block m = x_l[block m] @ (Wr_hi + 2^-7 Wr_lo),
                into p_shard[pbuf] (fp8, scaled by 2^-t_sh via host weights).
                Column halves so hi+lo PSUM pack into one 2KB bank per buf."""
                H = D // 2
                p_sb = spool.tile([P, D], F8, tag="psb", name=f"psb_{layer}_{m}")
                for h in range(2):
                    # one bank per half-pass: hi accumulates in cols 0:H,
                    # lo in cols H:2H, a single PSUM accumulation group
                    pp = psP.tile([P, D], F32, tag="pp",
                                  name=f"pp_{layer}_{m}_{h}")
                    cs = slice(h * H, (h + 1) * H)
                    for kk in range(KD // 2):
                        nc.tensor.matmul(
                            pp[:, 0:H],
                            lhsT=xt_src[:, 2 * kk:2 * kk + 2, m * P:(m + 1) * P],
                            rhs=wrh_sb[:, layer, 2 * kk:2 * kk + 2, cs],
                            start=(kk == 0), stop=False,
                            perf_mode=DR, skip_group_check=True)
                        nc.tensor.matmul(
                            pp[:, H:D],
                            lhsT=xt_src[:, 2 * kk:2 * kk + 2, m * P:(m + 1) * P],
                            rhs=wrl_sb[:, layer, 2 * kk:2 * kk + 2, cs],
                            start=False, stop=(kk == KD // 2 - 1),
                            perf_mode=DR, skip_group_check=True)
                    # combine: p = hi + 2^-7 * lo   (ACT scales lo, DVE adds)
                    lo_sb = spool.tile([P, H], F32, tag="losb",
                                       name=f"losb_{layer}_{m}_{h}")
                    nc.scalar.activation(
                        lo_sb[:], pp[:, H:D],
                        func=mybir.ActivationFunctionType.Copy,
                        scale=float(2.0 ** -LO_SHIFT))
                    nc.vector.tensor_tensor(p_sb[:, cs], pp[:, 0:H], lo_sb[:],
                                            op=ADD)
                nc.sync.dma_start(
                    p_shard[pbuf][m * P:(m + 1) * P, :], p_sb[:])

            def emit_ag(pbuf):
                if no_cc:
                    nc.sync.dma_start(
                        p_full[pbuf][:NC_NODES, :], p_shard[pbuf][:])
                else:
                    nc.gpsimd.collective_compute(
                        "AllGather", mybir.AluOpType.bypass, replica_groups=RG,
                        ins=[p_shard[pbuf][:]], outs=[p_full[pbuf][:]])

            # prologue: projection for layer 0
            for m in range(NBLK):
                emit_p_block(xt_cur, 0, m, 0)
            emit_ag(0)

            pool_ps = psS.tile([P, KD * G], F32, tag="pool", name="pool_ps")
            for l in range(n_layers):
                pbuf = l % 2
                xs_next = xpool.tile([P, NBLK, D], BF, tag="xs")
                last = l == NLAYERS - 1
                if not last:
                    xt_next = xtpool.tile([P, KD, NC_NODES], F8, tag="xt")
                for b in range(NBLK):
                    g = gpool.tile([P, T_pad, D], F8, tag="g")
                    if no_gather:
                        nc.vector.memset(g[:], 0)
                    else:
                        nsp = min(gsplit, T_pad)
                        th = (T_pad + nsp - 1) // nsp
                        col0 = b * (L // 16)
                        for s0 in range(0, T_pad, th):
                            s1 = min(s0 + th, T_pad)
                            nc.gpsimd.dma_gather(
                                g[:, s0:s1, :], p_full[pbuf][:],
                                idxe_sb[:, col0 + s0 * 8:col0 + s1 * 8],
                                (s1 - s0) * P, (s1 - s0) * P, D,
                                single_packet=False,
                                queue_num=b % nqueues)
                    aps = psA.tile([P, D], F32, tag="aps")
                    # Ws root first: it only needs resident data, so PE
                    # progresses on this block while its gather drains
                    for kk in range(KD // 2):
                        nc.tensor.matmul(
                            aps[:],
                            lhsT=xt_cur[:, 2 * kk:2 * kk + 2, b * P:(b + 1) * P],
                            rhs=ws_sb[:, l, 2 * kk:2 * kk + 2, :],
                            start=(kk == 0), stop=False,
                            perf_mode=DR)
                    if not bias_zero:
                        nc.tensor.matmul(
                            aps[:], lhsT=ones_sb[:], rhs=bias_sb[:, l, :],
                            start=False, stop=False)
                    for tp in range(NPAIR):
                        nc.tensor.matmul(
                            aps[:],
                            lhsT=oh_pair(b, 2 * tp),
                            rhs=g[:, 2 * tp:2 * tp + 2, :],
                            start=False,
                            stop=(TAIL == 0 and tp == NPAIR - 1),
                            perf_mode=DR)
                    if TAIL:
                        nc.tensor.matmul(
                            aps[:],
                            lhsT=oh_sb[:, (b * T_pad + T_pad - 1) * P:
                                       (b * T_pad + T_pad) * P],
                            rhs=g[:, T_pad - 1, :],
                            start=False, stop=True)
                    if last:
                        nc.vector.tensor_tensor(
                            xs_next[:, b, :], aps[:], xs_cur[:, b, :], op=ADD)
                        # pooling partials for this block, interleaved so they
                        # hide under later blocks' gathers
                        for j in range(KD):
                            nc.tensor.matmul(
                                pool_ps[:, j * G:(j + 1) * G],
                                lhsT=xs_next[:, b, j * P:(j + 1) * P],
                                rhs=goh_sb[:, b * G:(b + 1) * G],
                                start=(b == 0 and j == 0),
                                stop=(b == NBLK - 1 and j == KD - 1),
                                skip_group_check=True)
                    else:
                        t1 = spool.tile([P, D], BF, tag="t1")
                        nc.vector.tensor_tensor(
                            t1[:], aps[:], xs_cur[:, b, :], op=ADD)
                        nc.scalar.activation(
                            xs_next[:, b, :], t1[:],
                            func=mybir.ActivationFunctionType.Relu)
                        # transpose new block into xt_next (channel-major,
                        # fp8 at the next layer's x-scale)
                        sc_next = float(2.0 ** -S[l + 1])
                        for j in range(KD):
                            trps = psT.tile([P, P], BF, tag="tr")
                            nc.tensor.transpose(
                                trps[:], xs_next[:, b, j * P:(j + 1) * P],
                                ident_sb[:])
                            if S[l + 1] == 0:
                                nc.vector.tensor_copy(
                                    xt_next[:, j, b * P:(b + 1) * P], trps[:])
                            else:
                                nc.vector.tensor_scalar_mul(
                                    xt_next[:, j, b * P:(b + 1) * P], trps[:],
                                    sc_next)
                        # pipelined projection for layer l+1, block b
                        emit_p_block(xt_next, l + 1, b, 1 - pbuf)
                if not last:
                    emit_ag(1 - pbuf)
                    xt_cur = xt_next
                xs_cur = xs_next

            # ---- pooling partials were accumulated inside the last layer's
            # block loop (one PSUM region per 128-channel chunk)
            pool_sb = spool.tile([P, KD * G], F32, tag="pool_sb")
            nc.vector.tensor_copy(pool_sb[:], pool_ps[:])
            nc.sync.dma_start(pool_in[:], pool_sb[:])
            if no_cc:
                nc.sync.dma_start(pool_out[:], pool_sb[:])
            else:
                nc.gpsimd.collective_compute(
                    "AllReduce", ADD, replica_groups=RG,
                    ins=[pool_in[:]], outs=[pool_out[:]])
            pool2 = spool.tile([P, KD * G], F32, tag="pool2")
            nc.sync.dma_start(pool2[:], pool_out[:])
            poolbf = spool.tile([P, KD * G], BF, tag="poolbf")
            nc.vector.tensor_tensor(poolbf[:], pool2[:], invt_sb[:], op=MUL)
            fin_ps = psS.tile([P, G], F32, tag="fin", name="fin_ps")
            for k in range(KD):
                nc.tensor.matmul(
                    fin_ps[:], lhsT=wlin_sb[:, k, :],
                    rhs=poolbf[:, k * G:(k + 1) * G],
                    start=(k == 0), stop=(k == KD - 1))
            fin_sb = spool.tile([OUT, G], F32, tag="fin_sb")
            nc.vector.tensor_tensor(
                fin_sb[:], fin_ps[:], blin_sb[:, :1].to_broadcast([OUT, G]),
                op=ADD)
            nc.sync.dma_start(out_d[:], fin_sb[:])

    nc.compile()
    return nc


def kernel(**inputs):
    import os
    from concourse.bass_utils import run_bass_kernel_spmd

    per_core, shared, meta = _prep(inputs)
    nc = _build(meta)
    in_maps = [{**pc, **shared} for pc in per_core]
    trace = bool(int(os.environ.get("GCN_TRACE", "0")))
    res = run_bass_kernel_spmd(nc, in_maps, core_ids=list(range(NCORES)),
                               trace=trace)
    if trace:
        print(f"HW exec time: {res.exec_time_ns} ns")
        if res.instructions_and_trace is not None:
            print("trace:", res.instructions_and_trace[1])
    out_t = res.results[0]["out_t"]
    return np.ascontiguousarray(out_t.T).astype(np.float32)


# revision 12
# speedup vs baseline: 1.0225x; 1.0225x over previous
"""Distributed GCN (5x GraphConv(add) + residual/ReLU + mean-pool + linear)
for 8 Trainium2 NeuronCores.

Sharding: nodes partitioned contiguously across cores (1280 nodes/core, padded
to 10240). Each core owns the edges whose *destination* lands in its shard.
Aggregation is computed as A@(x@Wr): project first (p = x@Wr), AllGather p,
gather p[src[e]] rows with SWDGE dma_gather, then reduce edge tiles onto
destination nodes with one-hot segment matmuls on the tensor engine.

fp8 pipeline: p is stored/gathered as fp8e4m3 (halves gather DMA bytes), and
all heavy matmuls run in fp8 DoubleRow perf mode (2 K-slabs per instruction,
0.5 cycles/row): segment one-hot pairs, Ws root pairs, and Wr projection as an
fp8 hi + 2^7-scaled-lo pair (the lo PSUM is recombined at 2^-7 so the
effective Wr precision exceeds bf16). Per-layer power-of-2 x-scales (computed
from a host forward pass) keep activations inside fp8e4m3 range; they are
folded into the host-prepped weights so no extra device ops are needed except
a scaled copy when building the transposed x. The residual stream, pooling,
and final linear stay bf16/fp32.
"""

import numpy as np
import ml_dtypes

BF16 = ml_dtypes.bfloat16
FP8 = ml_dtypes.float8_e4m3      # TRN float8e4 (max normal 240)

N, E, D, OUT, G = 10000, 160000, 512, 128, 64
NCORES, P = 8, 128
NBLK = 10                     # 128-node blocks per core
NC_NODES = NBLK * P           # 1280
NPAD = NCORES * NC_NODES      # 10240
NLAYERS = 5
KD = D // P                   # 4 chunks of in-channels
LO_SHIFT = 7                  # Wr lo-residual stored scaled by 2^LO_SHIFT


def _wrap_idx(a):
    """[L] ints -> [128, L//16] int16 SWDGE index layout (16-partition wrap,
    replicated for the 8 Q7 cores)."""
    L = len(a)
    w = a.astype(np.int16).reshape(L // 16, 16).T
    return np.ascontiguousarray(np.tile(w, (8, 1)))


def _forward_stats(x, src, dst, batch, inputs):
    """fp32 host forward pass -> per-layer max|x_in| and max|p|."""
    try:
        import scipy.sparse as sp
        A = sp.csr_matrix((np.ones(len(src), np.float32), (dst, src)), shape=(N, N))
        segsum = lambda v: A @ v
    except Exception:
        def segsum(v):
            out = np.zeros((N, v.shape[1]), np.float32)
            np.add.at(out, dst, v[src])
            return out
    xmax, pmax = [], []
    xx = x
    for l in range(NLAYERS):
        Wr = np.asarray(inputs[f"Wr{l+1}"], np.float32)
        Ws = np.asarray(inputs[f"Ws{l+1}"], np.float32)
        b = np.asarray(inputs[f"b{l+1}"], np.float32)
        xmax.append(float(np.abs(xx).max()))
        p = xx @ Wr
        pmax.append(float(np.abs(p).max()))
        val = segsum(p) + xx @ Ws + b + xx
        if l < NLAYERS - 1:
            val = np.maximum(val, 0)
        xx = val
    return xmax, pmax


def _prep(inputs):
    x = np.asarray(inputs["x"], np.float32)
    ei = np.asarray(inputs["edge_index"]).astype(np.int64)
    batch = np.asarray(inputs["batch"]).astype(np.int64)
    src, dst = ei[0], ei[1]

    xmax, pmax = _forward_stats(x, src, dst, batch, inputs)
    # x scale: keep x * 2^-S[l] <= ~200 (fp8e4m3 max normal 240)
    S = [max(0, int(np.ceil(np.log2(m / 200.0)))) if m > 200 else 0 for m in xmax]
    # p scale: keep p * 2^-t <= ~200 (one-hot value 2^t restores it)
    pm = max(pmax[l] * 2.0 ** 0 for l in range(NLAYERS))
    t_sh = max(0, int(np.ceil(np.log2(pm / 200.0)))) if pm > 200 else 0

    order = np.argsort(dst, kind="stable")
    ds_, ss_ = dst[order], src[order]
    starts = np.searchsorted(ds_, np.arange(0, NPAD + 1, P))
    counts = np.diff(starts)
    T_pad = max(1, int(np.ceil(counts.max() / P)))
    L = T_pad * P

    xp = np.zeros((NPAD, D), np.float32)
    xp[:N] = x

    counts_g = np.bincount(batch, minlength=G)[:G]
    inv = (1.0 / np.maximum(counts_g, 1.0)).astype(np.float32)

    oh_val = float(2.0 ** t_sh)
    per_core = []
    for c in range(NCORES):
        idx_blocks = []
        oh_flat = np.zeros((P, NBLK * T_pad * P), FP8)
        goh = np.zeros((P, NBLK * G), BF16)
        for b in range(NBLK):
            gb = c * NBLK + b
            lo = gb * P
            s0, s1 = int(starts[gb]), int(starts[gb + 1])
            n = s1 - s0
            srcs = np.zeros(L, np.int64)
            srcs[:n] = ss_[s0:s1]
            dloc = ds_[s0:s1] - lo
            oh = np.zeros((L, P), FP8)
            oh[np.arange(n), dloc] = oh_val
            idx_blocks.append(_wrap_idx(srcs))
            oh_flat[:, b * T_pad * P:(b + 1) * T_pad * P] = (
                oh.reshape(T_pad, P, P).transpose(1, 0, 2).reshape(P, T_pad * P))
            nodes = lo + np.arange(P)
            valid = nodes < N
            goh[valid, b * G + batch[nodes[valid]]] = 1

        shard = xp[c * NC_NODES:(c + 1) * NC_NODES].astype(BF16)
        xt0 = np.ascontiguousarray(
            (shard.astype(np.float32) * 2.0 ** -S[0]).astype(FP8)
            .T.reshape(KD, P, NC_NODES).transpose(1, 0, 2))
        per_core.append(dict(
            x_shard=np.ascontiguousarray(shard),
            xt0=xt0,
            ohot=oh_flat,
            idxe=np.ascontiguousarray(np.concatenate(idx_blocks, axis=1)),
            goh=goh,
        ))

    wr_hi = np.zeros((P, NLAYERS, KD, D), FP8)
    wr_lo = np.zeros((P, NLAYERS, KD, D), FP8)
    ws_q = np.zeros((P, NLAYERS, KD, D), FP8)
    bias = np.zeros((P, NLAYERS, D), BF16)
    bias_zero = True
    for l in range(NLAYERS):
        WR = np.asarray(inputs[f"Wr{l+1}"], np.float32) * 2.0 ** (S[l] - t_sh)
        hi = WR.astype(FP8)
        lo = ((WR - hi.astype(np.float32)) * 2.0 ** LO_SHIFT).astype(FP8)
        WS = (np.asarray(inputs[f"Ws{l+1}"], np.float32) * 2.0 ** S[l]).astype(FP8)
        for arr, dst_arr in ((hi, wr_hi), (lo, wr_lo), (WS, ws_q)):
            dst_arr[:, l] = arr.reshape(KD, P, D).transpose(1, 0, 2)
        b_l = np.asarray(inputs[f"b{l+1}"], np.float32)
        if np.any(b_l != 0):
            bias_zero = False
        bias[0, l] = b_l.astype(BF16)
    ones_e0 = np.zeros((P, P), BF16)
    ones_e0[0, :] = 1
    wlin = np.ascontiguousarray(
        np.asarray(inputs["Wlin"], np.float32).reshape(KD, P, OUT)
        .transpose(1, 0, 2).astype(BF16))
    blin = np.asarray(inputs["blin"], np.float32).reshape(OUT, 1).astype(np.float32)
    shared = dict(
        wr_hi=wr_hi, wr_lo=wr_lo, ws=ws_q, bias=bias, ones=ones_e0,
        wlin=wlin, blin=blin,
        invt=np.ascontiguousarray(np.tile(inv, (P, KD)).astype(np.float32)),
        ident=np.eye(P, dtype=BF16),
    )
    meta = dict(T_pad=T_pad, S=S, t_sh=t_sh, bias_zero=bias_zero)
    return per_core, shared, meta


def _unwrap(w, L):
    """inverse of _wrap_idx: [128, L//16] -> [L]"""
    return np.ascontiguousarray(w[:16].T).reshape(-1)[:L].astype(np.int64)


def emulate(inputs):
    """Numpy emulation of the exact device dataflow (fp8/bf16 casts included).
    Validates the host-side scale/index/one-hot bookkeeping."""
    per_core, shared, meta = _prep(inputs)
    T_pad, S, t_sh = meta["T_pad"], meta["S"], meta["t_sh"]
    L = T_pad * P
    f32 = np.float32

    xs = [pc["x_shard"].astype(f32) for pc in per_core]       # [1280, 512]
    xts = [pc["xt0"] for pc in per_core]                      # fp8 [P, KD, NC]
    for l in range(NLAYERS):
        wr_hi = np.concatenate([shared["wr_hi"][:, l, k, :] for k in range(KD)],
                               axis=0).astype(f32)
        wr_lo = np.concatenate([shared["wr_lo"][:, l, k, :] for k in range(KD)],
                               axis=0).astype(f32)
        ws_l = np.concatenate([shared["ws"][:, l, k, :] for k in range(KD)],
                              axis=0).astype(f32)
        b_l = shared["bias"][0, l].astype(f32)
        p_parts = []
        for c in range(NCORES):
            xm = np.concatenate(
                [xts[c][:, k, :].astype(f32) for k in range(KD)], axis=0).T
            p = xm @ wr_hi + (xm @ wr_lo) * 2.0 ** -LO_SHIFT
            p_parts.append(p.astype(FP8).astype(f32))
        p_full = np.concatenate(p_parts, axis=0)              # [10240, 512]
        new_xs, new_xts = [], []
        for c in range(NCORES):
            nx = np.zeros((NC_NODES, D), f32)
            nxt = np.zeros((P, KD, NC_NODES), FP8)
            xm_all = np.concatenate(
                [xts[c][:, k, :].astype(f32) for k in range(KD)], axis=0).T
            for b in range(NBLK):
                idx = _unwrap(
                    per_core[c]["idxe"][:, b * (L // 16):(b + 1) * (L // 16)], L)
                gath = p_full[idx]                             # [L, 512]
                acc = np.zeros((P, D), f32)
                for t in range(T_pad):
                    oh = per_core[c]["ohot"][
                        :, (b * T_pad + t) * P:(b * T_pad + t + 1) * P
                    ].astype(f32)                             # [128e, 128d]
                    acc += oh.T @ gath[t * P:(t + 1) * P]
                acc += xm_all[b * P:(b + 1) * P] @ ws_l + b_l
                val = acc + xs[c][b * P:(b + 1) * P]
                if l < NLAYERS - 1:
                    val = np.maximum(val, 0)
                val = val.astype(BF16).astype(f32)
                nx[b * P:(b + 1) * P] = val
                if l < NLAYERS - 1:
                    sc = 2.0 ** -S[l + 1]
                    nxt[:, :, b * P:(b + 1) * P] = (
                        (val * sc).astype(FP8).T.reshape(KD, P, P)
                        .transpose(1, 0, 2))
            new_xs.append(nx)
            new_xts.append(nxt)
        xs, xts = new_xs, new_xts
    # pooling
    pooled_T = np.zeros((D, G), f32)
    for c in range(NCORES):
        goh = per_core[c]["goh"].astype(f32)
        for b in range(NBLK):
            blk = xs[c][b * P:(b + 1) * P].astype(BF16).astype(f32)
            for j in range(KD):
                pooled_T[j * P:(j + 1) * P] += (
                    blk[:, j * P:(j + 1) * P].T @ goh[:, b * G:(b + 1) * G])
    inv = shared["invt"][0, :G].astype(f32)
    pooled_T = (pooled_T * inv[None, :]).astype(BF16).astype(f32)
    wlin = np.concatenate([shared["wlin"][:, k, :] for k in range(KD)],
                          axis=0).astype(f32)                 # [512, 128]
    out_T = wlin.T @ pooled_T + shared["blin"][:, :1]         # [128, 64]
    return np.ascontiguousarray(out_T.T).astype(np.float32)


def _build(meta, enable_asserts=False):
    import os
    T_pad = meta["T_pad"]
    S, bias_zero = meta["S"], meta["bias_zero"]
    n_layers = int(os.environ.get("GCN_LAYERS", NLAYERS))
    no_gather = bool(int(os.environ.get("GCN_NO_GATHER", "0")))
    no_cc = bool(int(os.environ.get("GCN_NO_CC", "0")))
    bP, bA, bT = (int(v) for v in os.environ.get("GCN_BANKS", "2,2,2").split(","))
    gbufs = int(os.environ.get("GCN_GBUFS", "4"))
    gsplit = int(os.environ.get("GCN_GSPLIT", "2"))
    # SWDGE ring: big enough for 2 block-gathers so descriptor generation for
    # block b+1 overlaps block b's DMA drain; 2 queues decouple them further
    scratch = int(os.environ.get("GCN_SCRATCH", "16384"))
    nqueues = int(os.environ.get("GCN_NQUEUES", "2"))
    import concourse.bass as bass
    import concourse.mybir as mybir
    import concourse.tile as tile
    from concourse import bacc

    F32 = mybir.dt.float32
    BF = mybir.dt.bfloat16
    F8 = mybir.dt.float8e4
    I16 = mybir.dt.int16
    ADD = mybir.AluOpType.add
    MUL = mybir.AluOpType.mult
    DR = mybir.MatmulPerfMode.DoubleRow
    L = T_pad * P
    RG = [list(range(NCORES))]
    NPAIR, TAIL = T_pad // 2, T_pad % 2

    nc = bacc.Bacc("TRN2", target_bir_lowering=False, debug=False,
                   enable_asserts=enable_asserts, num_devices=NCORES,
                   dynamic_dma_scratch_size=scratch,
                   num_swdge_queues=nqueues)

    # per-core inputs
    x_d = nc.dram_tensor("x_shard", [NC_NODES, D], BF, kind="ExternalInput")
    xt0_d = nc.dram_tensor("xt0", [P, KD, NC_NODES], F8, kind="ExternalInput")
    oh_d = nc.dram_tensor("ohot", [P, NBLK * T_pad * P], F8, kind="ExternalInput")
    idxe_d = nc.dram_tensor("idxe", [P, NBLK * (L // 16)], I16, kind="ExternalInput")
    goh_d = nc.dram_tensor("goh", [P, NBLK * G], BF, kind="ExternalInput")
    # shared inputs
    wrh_d = nc.dram_tensor("wr_hi", [P, NLAYERS, KD, D], F8, kind="ExternalInput")
    wrl_d = nc.dram_tensor("wr_lo", [P, NLAYERS, KD, D], F8, kind="ExternalInput")
    ws_d = nc.dram_tensor("ws", [P, NLAYERS, KD, D], F8, kind="ExternalInput")
    bias_d = nc.dram_tensor("bias", [P, NLAYERS, D], BF, kind="ExternalInput")
    ones_d = nc.dram_tensor("ones", [P, P], BF, kind="ExternalInput")
    wlin_d = nc.dram_tensor("wlin", [P, KD, OUT], BF, kind="ExternalInput")
    blin_d = nc.dram_tensor("blin", [OUT, 1], F32, kind="ExternalInput")
    invt_d = nc.dram_tensor("invt", [P, KD * G], F32, kind="ExternalInput")
    ident_d = nc.dram_tensor("ident", [P, P], BF, kind="ExternalInput")
    # internal DRAM (double-buffered by layer parity so the AllGather for
    # layer l+1 never WAR-depends on layer l's gathers)
    p_shard = [nc.dram_tensor(f"p_shard{i}", [NC_NODES, D], F8) for i in (0, 1)]
    p_full = [nc.dram_tensor(f"p_full{i}", [NPAD, D], F8, addr_space="Shared")
              for i in (0, 1)]
    pool_in = nc.dram_tensor("pool_in", [P, KD * G], F32)
    pool_out = nc.dram_tensor("pool_out", [P, KD * G], F32, addr_space="Shared")
    # output
    out_d = nc.dram_tensor("out_t", [OUT, G], F32, kind="ExternalOutput")

    with tile.TileContext(nc) as tc:
        with (
            tc.tile_pool(name="const", bufs=1) as const,
            tc.tile_pool(name="xs", bufs=2) as xpool,
            tc.tile_pool(name="xt", bufs=2) as xtpool,
            tc.tile_pool(name="gath", bufs=gbufs) as gpool,
            tc.tile_pool(name="small", bufs=int(os.environ.get("GCN_SBUFS", "4"))) as spool,
            tc.tile_pool(name="psP", bufs=bP, space="PSUM") as psP,
            tc.tile_pool(name="psA", bufs=bA, space="PSUM") as psA,
            tc.tile_pool(name="psS", bufs=1, space="PSUM") as psS,
            tc.tile_pool(name="psT", bufs=bT, space="PSUM") as psT,
        ):
            # ---- constants to SBUF
            oh_sb = const.tile([P, NBLK * T_pad * P], F8, tag="oh")
            nc.sync.dma_start(oh_sb[:], oh_d[:])
            idxe_sb = const.tile([P, NBLK * (L // 16)], I16, tag="idxe")
            nc.sync.dma_start(idxe_sb[:], idxe_d[:])
            ident_sb = const.tile([P, P], BF, tag="ident")
            nc.sync.dma_start(ident_sb[:], ident_d[:])
            goh_sb = const.tile([P, NBLK * G], BF, tag="goh")
            nc.sync.dma_start(goh_sb[:], goh_d[:])
            wrh_sb = const.tile([P, NLAYERS, KD, D], F8, tag="wrh")
            nc.sync.dma_start(wrh_sb[:], wrh_d[:])
            wrl_sb = const.tile([P, NLAYERS, KD, D], F8, tag="wrl")
            nc.sync.dma_start(wrl_sb[:], wrl_d[:])
            ws_sb = const.tile([P, NLAYERS, KD, D], F8, tag="ws")
            nc.sync.dma_start(ws_sb[:], ws_d[:])
            if not bias_zero:
                bias_sb = const.tile([P, NLAYERS, D], BF, tag="bias")
                nc.sync.dma_start(bias_sb[:], bias_d[:])
                ones_sb = const.tile([P, P], BF, tag="ones")
                nc.sync.dma_start(ones_sb[:], ones_d[:])
            wlin_sb = const.tile([P, KD, OUT], BF, tag="wlin")
            nc.sync.dma_start(wlin_sb[:], wlin_d[:])
            blin_sb = const.tile([OUT, 1], F32, tag="blin")
            nc.sync.dma_start(blin_sb[:], blin_d[:])
            invt_sb = const.tile([P, KD * G], F32, tag="invt")
            nc.sync.dma_start(invt_sb[:], invt_d[:])

            xs_cur = xpool.tile([P, NBLK, D], BF, tag="xs")
            nc.sync.dma_start(xs_cur[:], x_d.ap().rearrange("(b p) d -> p b d", p=P))
            xt_cur = xtpool.tile([P, KD, NC_NODES], F8, tag="xt")
            nc.sync.dma_start(xt_cur[:], xt0_d[:])

            def oh_pair(b, t):
                return oh_sb[:].rearrange("p (n q) -> p n q", q=P)[
                    :, b * T_pad + t:b * T_pad + t + 2, :]

            def emit_p_block(xt_src, layer, m, pbuf):
                """p[l=layer] block m = x_l[block m] @ (Wr_hi + 2^-7 Wr_lo),
                into p_shard[pbuf] (fp8, scaled by 2^-t_sh via host weights).
                Column halves so hi+lo PSUM pack into one 2KB bank per buf."""
                H = D // 2
                p_sb = spool.tile([P, D], F8, tag="psb", name=f"psb_{layer}_{m}")
                for h in range(2):
                    # one bank per half-pass: hi accumulates in cols 0:H,
                    # lo in cols H:2H, a single PSUM accumulation group
                    pp = psP.tile([P, D], F32, tag="pp",
                                  name=f"pp_{layer}_{m}_{h}")
                    cs = slice(h * H, (h + 1) * H)
                    for kk in range(KD // 2):
                        nc.tensor.matmul(
                            pp[:, 0:H],
                            lhsT=xt_src[:, 2 * kk:2 * kk + 2, m * P:(m + 1) * P],
                            rhs=wrh_sb[:, layer, 2 * kk:2 * kk + 2, cs],
                            start=(kk == 0), stop=False,
                            perf_mode=DR, skip_group_check=True)
                        nc.tensor.matmul(
                            pp[:, H:D],
                            lhsT=xt_src[:, 2 * kk:2 * kk + 2, m * P:(m + 1) * P],
                            rhs=wrl_sb[:, layer, 2 * kk:2 * kk + 2, cs],
                            start=False, stop=(kk == KD // 2 - 1),
                            perf_mode=DR, skip_group_check=True)
                    # combine: p = hi + 2^-7 * lo   (ACT scales lo, DVE adds)
                    lo_sb = spool.tile([P, H], F32, tag="losb",
                                       name=f"losb_{layer}_{m}_{h}")
                    nc.scalar.activation(
                        lo_sb[:], pp[:, H:D],
                        func=mybir.ActivationFunctionType.Copy,
                        scale=float(2.0 ** -LO_SHIFT))
                    nc.vector.tensor_tensor(p_sb[:, cs], pp[:, 0:H], lo_sb[:],
                                            op=ADD)
                nc.sync.dma_start(
                    p_shard[pbuf][m * P:(m + 1) * P, :], p_sb[:])

            def emit_ag(pbuf):
                if no_cc:
                    nc.sync.dma_start(
                        p_full[pbuf][:NC_NODES, :], p_shard[pbuf][:])
                else:
                    nc.gpsimd.collective_compute(
                        "AllGather", mybir.AluOpType.bypass, replica_groups=RG,
                        ins=[p_shard[pbuf][:]], outs=[p_full[pbuf][:]])

            # prologue: projection for layer 0
            for m in range(NBLK):
                emit_p_block(xt_cur, 0, m, 0)
            emit_ag(0)

            pool_ps = psS.tile([P, KD * G], F32, tag="pool", name="pool_ps")
            for l in range(n_layers):
                pbuf = l % 2
                xs_next = xpool.tile([P, NBLK, D], BF, tag="xs")
                last = l == NLAYERS - 1
                if not last:
                    xt_next = xtpool.tile([P, KD, NC_NODES], F8, tag="xt")
                for b in range(NBLK):
                    g = gpool.tile([P, T_pad, D], F8, tag="g")
                    if no_gather:
                        nc.vector.memset(g[:], 0)
                    else:
                        nsp = min(gsplit, T_pad)
                        th = (T_pad + nsp - 1) // nsp
                        col0 = b * (L // 16)
                        for s0 in range(0, T_pad, th):
                            s1 = min(s0 + th, T_pad)
                            nc.gpsimd.dma_gather(
                                g[:, s0:s1, :], p_full[pbuf][:],
                                idxe_sb[:, col0 + s0 * 8:col0 + s1 * 8],
                                (s1 - s0) * P, (s1 - s0) * P, D,
                                single_packet=False,
                                queue_num=b % nqueues)
                    aps = psA.tile([P, D], F32, tag="aps")
                    # Ws root first: it only needs resident data, so PE
                    # progresses on this block while its gather drains
                    for kk in range(KD // 2):
                        nc.tensor.matmul(
                            aps[:],
                            lhsT=xt_cur[:, 2 * kk:2 * kk + 2, b * P:(b + 1) * P],
                            rhs=ws_sb[:, l, 2 * kk:2 * kk + 2, :],
                            start=(kk == 0), stop=False,
                            perf_mode=DR)
                    if not bias_zero:
                        nc.tensor.matmul(
                            aps[:], lhsT=ones_sb[:], rhs=bias_sb[:, l, :],
                            start=False, stop=False)
                    for tp in range(NPAIR):
                        nc.tensor.matmul(
                            aps[:],
                            lhsT=oh_pair(b, 2 * tp),
                            rhs=g[:, 2 * tp:2 * tp + 2, :],
                            start=False,
                            stop=(TAIL == 0 and tp == NPAIR - 1),
                            perf_mode=DR)
                    if TAIL:
                        nc.tensor.matmul(
                            aps[:],
                            lhsT=oh_sb[:, (b * T_pad + T_pad - 1) * P:
                                       (b * T_pad + T_pad) * P],
                            rhs=g[:, T_pad - 1, :],
                            start=False, stop=True)
                    if last:
                        nc.vector.tensor_tensor(
                            xs_next[:, b, :], aps[:], xs_cur[:, b, :], op=ADD)
                        # pooling partials for this block, interleaved so they
                        # hide under later blocks' gathers
                        for j in range(KD):
                            nc.tensor.matmul(
                                pool_ps[:, j * G:(j + 1) * G],
                                lhsT=xs_next[:, b, j * P:(j + 1) * P],
                                rhs=goh_sb[:, b * G:(b + 1) * G],
                                start=(b == 0 and j == 0),
                                stop=(b == NBLK - 1 and j == KD - 1),
                                skip_group_check=True)
                    else:
                        t1 = spool.tile([P, D], BF, tag="t1")
                        nc.vector.tensor_tensor(
                            t1[:], aps[:], xs_cur[:, b, :], op=ADD)
                        nc.scalar.activation(
                            xs_next[:, b, :], t1[:],
                            func=mybir.ActivationFunctionType.Relu)
                        # transpose new block into xt_next (channel-major,
                        # fp8 at the next layer's x-scale): all 4 chunks into
                        # one PSUM bank as a single group, then ONE DVE copy
                        sc_next = float(2.0 ** -S[l + 1])
                        trp = psT.tile([P, KD * P], BF, tag="tr")
                        for j in range(KD):
                            nc.tensor.matmul(
                                trp[:, j * P:(j + 1) * P],
                                lhsT=xs_next[:, b, j * P:(j + 1) * P],
                                rhs=ident_sb[:],
                                is_transpose=True,
                                start=(j == 0), stop=(j == KD - 1),
                                skip_group_check=True)
                        trv = trp[:].rearrange("p (j q) -> p j q", q=P)
                        if S[l + 1] == 0:
                            nc.vector.tensor_copy(
                                xt_next[:, :, b * P:(b + 1) * P], trv)
                        else:
                            nc.vector.tensor_scalar_mul(
                                xt_next[:, :, b * P:(b + 1) * P], trv, sc_next)
                        # pipelined projection for layer l+1, block b
                        emit_p_block(xt_next, l + 1, b, 1 - pbuf)
                if not last:
                    emit_ag(1 - pbuf)
                    xt_cur = xt_next
                xs_cur = xs_next

            # ---- pooling partials were accumulated inside the last layer's
            # block loop (one PSUM region per 128-channel chunk)
            pool_sb = spool.tile([P, KD * G], F32, tag="pool_sb")
            nc.vector.tensor_copy(pool_sb[:], pool_ps[:])
            nc.sync.dma_start(pool_in[:], pool_sb[:])
            if no_cc:
                nc.sync.dma_start(pool_out[:], pool_sb[:])
            else:
                nc.gpsimd.collective_compute(
                    "AllReduce", ADD, replica_groups=RG,
                    ins=[pool_in[:]], outs=[pool_out[:]])
            pool2 = spool.tile([P, KD * G], F32, tag="pool2")
            nc.sync.dma_start(pool2[:], pool_out[:])
            poolbf = spool.tile([P, KD * G], BF, tag="poolbf")
            nc.vector.tensor_tensor(poolbf[:], pool2[:], invt_sb[:], op=MUL)
            fin_ps = psS.tile([P, G], F32, tag="fin", name="fin_ps")
            for k in range(KD):
                nc.tensor.matmul(
                    fin_ps[:], lhsT=wlin_sb[:, k, :],
                    rhs=poolbf[:, k * G:(k + 1) * G],
                    start=(k == 0), stop=(k == KD - 1))
            fin_sb = spool.tile([OUT, G], F32, tag="fin_sb")
            nc.vector.tensor_tensor(
                fin_sb[:], fin_ps[:], blin_sb[:, :1].to_broadcast([OUT, G]),
                op=ADD)
            nc.sync.dma_start(out_d[:], fin_sb[:])

    nc.compile()
    return nc


def kernel(**inputs):
    import os
    from concourse.bass_utils import run_bass_kernel_spmd

    per_core, shared, meta = _prep(inputs)
    nc = _build(meta)
    in_maps = [{**pc, **shared} for pc in per_core]
    trace = bool(int(os.environ.get("GCN_TRACE", "0")))
    res = run_bass_kernel_spmd(nc, in_maps, core_ids=list(range(NCORES)),
                               trace=trace)
    if trace:
        print(f"HW exec time: {res.exec_time_ns} ns")
        if res.instructions_and_trace is not None:
            print("trace:", res.instructions_and_trace[1])
    out_t = res.results[0]["out_t"]
    return np.ascontiguousarray(out_t.T).astype(np.float32)


# revision 13
# speedup vs baseline: 1.0441x; 1.0211x over previous
"""Distributed GCN (5x GraphConv(add) + residual/ReLU + mean-pool + linear)
for 8 Trainium2 NeuronCores.

Sharding: nodes partitioned contiguously across cores (1280 nodes/core, padded
to 10240). Each core owns the edges whose *destination* lands in its shard.
Aggregation is computed as A@(x@Wr): project first (p = x@Wr), AllGather p,
gather p[src[e]] rows with SWDGE dma_gather, then reduce edge tiles onto
destination nodes with one-hot segment matmuls on the tensor engine.

fp8 pipeline: p is stored/gathered as fp8e4m3 (halves gather DMA bytes), and
all heavy matmuls run in fp8 DoubleRow perf mode (2 K-slabs per instruction,
0.5 cycles/row): segment one-hot pairs, Ws root pairs, and Wr projection as an
fp8 hi + 2^7-scaled-lo pair (the lo PSUM is recombined at 2^-7 so the
effective Wr precision exceeds bf16). Per-layer power-of-2 x-scales (computed
from a host forward pass) keep activations inside fp8e4m3 range; they are
folded into the host-prepped weights so no extra device ops are needed except
a scaled copy when building the transposed x. The residual stream, pooling,
and final linear stay bf16/fp32.
"""

import numpy as np
import ml_dtypes

BF16 = ml_dtypes.bfloat16
FP8 = ml_dtypes.float8_e4m3      # TRN float8e4 (max normal 240)

N, E, D, OUT, G = 10000, 160000, 512, 128, 64
NCORES, P = 8, 128
NBLK = 10                     # 128-node blocks per core
NC_NODES = NBLK * P           # 1280
NPAD = NCORES * NC_NODES      # 10240
NLAYERS = 5
KD = D // P                   # 4 chunks of in-channels
LO_SHIFT = 7                  # Wr lo-residual stored scaled by 2^LO_SHIFT


def _wrap_idx(a):
    """[L] ints -> [128, L//16] int16 SWDGE index layout (16-partition wrap,
    replicated for the 8 Q7 cores)."""
    L = len(a)
    w = a.astype(np.int16).reshape(L // 16, 16).T
    return np.ascontiguousarray(np.tile(w, (8, 1)))


def _forward_stats(x, src, dst, batch, inputs):
    """fp32 host forward pass -> per-layer max|x_in| and max|p|."""
    try:
        import scipy.sparse as sp
        A = sp.csr_matrix((np.ones(len(src), np.float32), (dst, src)), shape=(N, N))
        segsum = lambda v: A @ v
    except Exception:
        def segsum(v):
            out = np.zeros((N, v.shape[1]), np.float32)
            np.add.at(out, dst, v[src])
            return out
    xmax, pmax = [], []
    xx = x
    for l in range(NLAYERS):
        Wr = np.asarray(inputs[f"Wr{l+1}"], np.float32)
        Ws = np.asarray(inputs[f"Ws{l+1}"], np.float32)
        b = np.asarray(inputs[f"b{l+1}"], np.float32)
        xmax.append(float(np.abs(xx).max()))
        p = xx @ Wr
        pmax.append(float(np.abs(p).max()))
        val = segsum(p) + xx @ Ws + b + xx
        if l < NLAYERS - 1:
            val = np.maximum(val, 0)
        xx = val
    return xmax, pmax


def _prep(inputs):
    x = np.asarray(inputs["x"], np.float32)
    ei = np.asarray(inputs["edge_index"]).astype(np.int64)
    batch = np.asarray(inputs["batch"]).astype(np.int64)
    src, dst = ei[0], ei[1]

    xmax, pmax = _forward_stats(x, src, dst, batch, inputs)
    # x scale: keep x * 2^-S[l] <= ~200 (fp8e4m3 max normal 240)
    S = [max(0, int(np.ceil(np.log2(m / 200.0)))) if m > 200 else 0 for m in xmax]
    # p scale: keep p * 2^-t <= ~200 (one-hot value 2^t restores it)
    pm = max(pmax[l] * 2.0 ** 0 for l in range(NLAYERS))
    t_sh = max(0, int(np.ceil(np.log2(pm / 200.0)))) if pm > 200 else 0

    order = np.argsort(dst, kind="stable")
    ds_, ss_ = dst[order], src[order]
    starts = np.searchsorted(ds_, np.arange(0, NPAD + 1, P))
    counts = np.diff(starts)
    T_pad = max(1, int(np.ceil(counts.max() / P)))
    L = T_pad * P

    xp = np.zeros((NPAD, D), np.float32)
    xp[:N] = x

    counts_g = np.bincount(batch, minlength=G)[:G]
    inv = (1.0 / np.maximum(counts_g, 1.0)).astype(np.float32)

    oh_val = float(2.0 ** t_sh)
    per_core = []
    for c in range(NCORES):
        idx_blocks = []
        oh_flat = np.zeros((P, NBLK * T_pad * P), FP8)
        goh = np.zeros((P, NBLK * G), BF16)
        for b in range(NBLK):
            gb = c * NBLK + b
            lo = gb * P
            s0, s1 = int(starts[gb]), int(starts[gb + 1])
            n = s1 - s0
            srcs = np.zeros(L, np.int64)
            srcs[:n] = ss_[s0:s1]
            dloc = ds_[s0:s1] - lo
            oh = np.zeros((L, P), FP8)
            oh[np.arange(n), dloc] = oh_val
            idx_blocks.append(_wrap_idx(srcs))
            oh_flat[:, b * T_pad * P:(b + 1) * T_pad * P] = (
                oh.reshape(T_pad, P, P).transpose(1, 0, 2).reshape(P, T_pad * P))
            nodes = lo + np.arange(P)
            valid = nodes < N
            goh[valid, b * G + batch[nodes[valid]]] = 1

        shard = xp[c * NC_NODES:(c + 1) * NC_NODES].astype(BF16)
        xt0 = np.ascontiguousarray(
            (shard.astype(np.float32) * 2.0 ** -S[0]).astype(FP8)
            .T.reshape(KD, P, NC_NODES).transpose(1, 0, 2))
        per_core.append(dict(
            x_shard=np.ascontiguousarray(shard),
            xt0=xt0,
            ohot=oh_flat,
            idxe=np.ascontiguousarray(np.concatenate(idx_blocks, axis=1)),
            goh=goh,
        ))

    wr_hi = np.zeros((P, NLAYERS, KD, D), FP8)
    wr_lo = np.zeros((P, NLAYERS, KD, D), FP8)
    ws_q = np.zeros((P, NLAYERS, KD, D), FP8)
    bias = np.zeros((P, NLAYERS, D), BF16)
    bias_zero = True
    for l in range(NLAYERS):
        WR = np.asarray(inputs[f"Wr{l+1}"], np.float32) * 2.0 ** (S[l] - t_sh)
        hi = WR.astype(FP8)
        lo = ((WR - hi.astype(np.float32)) * 2.0 ** LO_SHIFT).astype(FP8)
        WS = (np.asarray(inputs[f"Ws{l+1}"], np.float32) * 2.0 ** S[l]).astype(FP8)
        for arr, dst_arr in ((hi, wr_hi), (lo, wr_lo), (WS, ws_q)):
            dst_arr[:, l] = arr.reshape(KD, P, D).transpose(1, 0, 2)
        b_l = np.asarray(inputs[f"b{l+1}"], np.float32)
        if np.any(b_l != 0):
            bias_zero = False
        bias[0, l] = b_l.astype(BF16)
    ones_e0 = np.zeros((P, P), BF16)
    ones_e0[0, :] = 1
    wlin = np.ascontiguousarray(
        np.asarray(inputs["Wlin"], np.float32).reshape(KD, P, OUT)
        .transpose(1, 0, 2).astype(BF16))
    blin = np.asarray(inputs["blin"], np.float32).reshape(OUT, 1).astype(np.float32)
    shared = dict(
        wr_hi=wr_hi, wr_lo=wr_lo, ws=ws_q, bias=bias, ones=ones_e0,
        wlin=wlin, blin=blin,
        invt=np.ascontiguousarray(np.tile(inv, (P, KD)).astype(np.float32)),
        ident=np.eye(P, dtype=BF16),
    )
    meta = dict(T_pad=T_pad, S=S, t_sh=t_sh, bias_zero=bias_zero)
    return per_core, shared, meta


def _unwrap(w, L):
    """inverse of _wrap_idx: [128, L//16] -> [L]"""
    return np.ascontiguousarray(w[:16].T).reshape(-1)[:L].astype(np.int64)


def emulate(inputs):
    """Numpy emulation of the exact device dataflow (fp8/bf16 casts included).
    Validates the host-side scale/index/one-hot bookkeeping."""
    per_core, shared, meta = _prep(inputs)
    T_pad, S, t_sh = meta["T_pad"], meta["S"], meta["t_sh"]
    L = T_pad * P
    f32 = np.float32

    xs = [pc["x_shard"].astype(f32) for pc in per_core]       # [1280, 512]
    xts = [pc["xt0"] for pc in per_core]                      # fp8 [P, KD, NC]
    for l in range(NLAYERS):
        wr_hi = np.concatenate([shared["wr_hi"][:, l, k, :] for k in range(KD)],
                               axis=0).astype(f32)
        wr_lo = np.concatenate([shared["wr_lo"][:, l, k, :] for k in range(KD)],
                               axis=0).astype(f32)
        ws_l = np.concatenate([shared["ws"][:, l, k, :] for k in range(KD)],
                              axis=0).astype(f32)
        b_l = shared["bias"][0, l].astype(f32)
        p_parts = []
        for c in range(NCORES):
            xm = np.concatenate(
                [xts[c][:, k, :].astype(f32) for k in range(KD)], axis=0).T
            p = xm @ wr_hi + (xm @ wr_lo) * 2.0 ** -LO_SHIFT
            p_parts.append(p.astype(FP8).astype(f32))
        p_full = np.concatenate(p_parts, axis=0)              # [10240, 512]
        new_xs, new_xts = [], []
        for c in range(NCORES):
            nx = np.zeros((NC_NODES, D), f32)
            nxt = np.zeros((P, KD, NC_NODES), FP8)
            xm_all = np.concatenate(
                [xts[c][:, k, :].astype(f32) for k in range(KD)], axis=0).T
            for b in range(NBLK):
                idx = _unwrap(
                    per_core[c]["idxe"][:, b * (L // 16):(b + 1) * (L // 16)], L)
                gath = p_full[idx]                             # [L, 512]
                acc = np.zeros((P, D), f32)
                for t in range(T_pad):
                    oh = per_core[c]["ohot"][
                        :, (b * T_pad + t) * P:(b * T_pad + t + 1) * P
                    ].astype(f32)                             # [128e, 128d]
                    acc += oh.T @ gath[t * P:(t + 1) * P]
                acc += xm_all[b * P:(b + 1) * P] @ ws_l + b_l
                val = acc + xs[c][b * P:(b + 1) * P]
                if l < NLAYERS - 1:
                    val = np.maximum(val, 0)
                val = val.astype(BF16).astype(f32)
                nx[b * P:(b + 1) * P] = val
                if l < NLAYERS - 1:
                    sc = 2.0 ** -S[l + 1]
                    nxt[:, :, b * P:(b + 1) * P] = (
                        (val * sc).astype(FP8).T.reshape(KD, P, P)
                        .transpose(1, 0, 2))
            new_xs.append(nx)
            new_xts.append(nxt)
        xs, xts = new_xs, new_xts
    # pooling
    pooled_T = np.zeros((D, G), f32)
    for c in range(NCORES):
        goh = per_core[c]["goh"].astype(f32)
        for b in range(NBLK):
            blk = xs[c][b * P:(b + 1) * P].astype(BF16).astype(f32)
            for j in range(KD):
                pooled_T[j * P:(j + 1) * P] += (
                    blk[:, j * P:(j + 1) * P].T @ goh[:, b * G:(b + 1) * G])
    inv = shared["invt"][0, :G].astype(f32)
    pooled_T = (pooled_T * inv[None, :]).astype(BF16).astype(f32)
    wlin = np.concatenate([shared["wlin"][:, k, :] for k in range(KD)],
                          axis=0).astype(f32)                 # [512, 128]
    out_T = wlin.T @ pooled_T + shared["blin"][:, :1]         # [128, 64]
    return np.ascontiguousarray(out_T.T).astype(np.float32)


def _build(meta, enable_asserts=False):
    import os
    T_pad = meta["T_pad"]
    S, bias_zero = meta["S"], meta["bias_zero"]
    n_layers = int(os.environ.get("GCN_LAYERS", NLAYERS))
    no_gather = bool(int(os.environ.get("GCN_NO_GATHER", "0")))
    no_cc = bool(int(os.environ.get("GCN_NO_CC", "0")))
    bP, bA, bT = (int(v) for v in os.environ.get("GCN_BANKS", "2,2,2").split(","))
    gbufs = int(os.environ.get("GCN_GBUFS", "4"))
    gsplit = int(os.environ.get("GCN_GSPLIT", "2"))
    # SWDGE ring: big enough for 2 block-gathers so descriptor generation for
    # block b+1 overlaps block b's DMA drain; 2 queues decouple them further
    scratch = int(os.environ.get("GCN_SCRATCH", "16384"))
    nqueues = int(os.environ.get("GCN_NQUEUES", "2"))
    import concourse.bass as bass
    import concourse.mybir as mybir
    import concourse.tile as tile
    from concourse import bacc

    F32 = mybir.dt.float32
    BF = mybir.dt.bfloat16
    F8 = mybir.dt.float8e4
    I16 = mybir.dt.int16
    ADD = mybir.AluOpType.add
    MUL = mybir.AluOpType.mult
    DR = mybir.MatmulPerfMode.DoubleRow
    L = T_pad * P
    RG = [list(range(NCORES))]
    NPAIR, TAIL = T_pad // 2, T_pad % 2

    nc = bacc.Bacc("TRN2", target_bir_lowering=False, debug=False,
                   enable_asserts=enable_asserts, num_devices=NCORES,
                   dynamic_dma_scratch_size=scratch,
                   num_swdge_queues=nqueues)

    # per-core inputs
    x_d = nc.dram_tensor("x_shard", [NC_NODES, D], BF, kind="ExternalInput")
    xt0_d = nc.dram_tensor("xt0", [P, KD, NC_NODES], F8, kind="ExternalInput")
    oh_d = nc.dram_tensor("ohot", [P, NBLK * T_pad * P], F8, kind="ExternalInput")
    idxe_d = nc.dram_tensor("idxe", [P, NBLK * (L // 16)], I16, kind="ExternalInput")
    goh_d = nc.dram_tensor("goh", [P, NBLK * G], BF, kind="ExternalInput")
    # shared inputs
    wrh_d = nc.dram_tensor("wr_hi", [P, NLAYERS, KD, D], F8, kind="ExternalInput")
    wrl_d = nc.dram_tensor("wr_lo", [P, NLAYERS, KD, D], F8, kind="ExternalInput")
    ws_d = nc.dram_tensor("ws", [P, NLAYERS, KD, D], F8, kind="ExternalInput")
    bias_d = nc.dram_tensor("bias", [P, NLAYERS, D], BF, kind="ExternalInput")
    ones_d = nc.dram_tensor("ones", [P, P], BF, kind="ExternalInput")
    wlin_d = nc.dram_tensor("wlin", [P, KD, OUT], BF, kind="ExternalInput")
    blin_d = nc.dram_tensor("blin", [OUT, 1], F32, kind="ExternalInput")
    invt_d = nc.dram_tensor("invt", [P, KD * G], F32, kind="ExternalInput")
    ident_d = nc.dram_tensor("ident", [P, P], BF, kind="ExternalInput")
    # internal DRAM (double-buffered by layer parity so the AllGather for
    # layer l+1 never WAR-depends on layer l's gathers)
    p_shard = [nc.dram_tensor(f"p_shard{i}", [NC_NODES, D], F8) for i in (0, 1)]
    p_full = [nc.dram_tensor(f"p_full{i}", [NPAD, D], F8, addr_space="Shared")
              for i in (0, 1)]
    pool_in = nc.dram_tensor("pool_in", [P, KD * G], F32)
    pool_out = nc.dram_tensor("pool_out", [P, KD * G], F32, addr_space="Shared")
    # output
    out_d = nc.dram_tensor("out_t", [OUT, G], F32, kind="ExternalOutput")

    with tile.TileContext(nc) as tc:
        with (
            tc.tile_pool(name="const", bufs=1) as const,
            tc.tile_pool(name="xs", bufs=2) as xpool,
            tc.tile_pool(name="xt", bufs=2) as xtpool,
            tc.tile_pool(name="gath", bufs=gbufs) as gpool,
            tc.tile_pool(name="small", bufs=int(os.environ.get("GCN_SBUFS", "4"))) as spool,
            tc.tile_pool(name="psP", bufs=bP, space="PSUM") as psP,
            tc.tile_pool(name="psA", bufs=bA, space="PSUM") as psA,
            tc.tile_pool(name="psS", bufs=1, space="PSUM") as psS,
            tc.tile_pool(name="psT", bufs=bT, space="PSUM") as psT,
        ):
            # ---- constants to SBUF (prologue-projection inputs first so
            # the PE starts while the big one-hot/index tables stream in)
            wrh_sb = const.tile([P, NLAYERS, KD, D], F8, tag="wrh")
            nc.sync.dma_start(wrh_sb[:], wrh_d[:])
            wrl_sb = const.tile([P, NLAYERS, KD, D], F8, tag="wrl")
            nc.sync.dma_start(wrl_sb[:], wrl_d[:])
            ident_sb = const.tile([P, P], BF, tag="ident")
            nc.sync.dma_start(ident_sb[:], ident_d[:])
            xs_cur = xpool.tile([P, NBLK, D], BF, tag="xs")
            nc.sync.dma_start(xs_cur[:], x_d.ap().rearrange("(b p) d -> p b d", p=P))
            xt_cur = xtpool.tile([P, KD, NC_NODES], F8, tag="xt")
            nc.sync.dma_start(xt_cur[:], xt0_d[:])
            oh_sb = const.tile([P, NBLK * T_pad * P], F8, tag="oh")
            nc.sync.dma_start(oh_sb[:], oh_d[:])
            idxe_sb = const.tile([P, NBLK * (L // 16)], I16, tag="idxe")
            nc.sync.dma_start(idxe_sb[:], idxe_d[:])
            goh_sb = const.tile([P, NBLK * G], BF, tag="goh")
            nc.sync.dma_start(goh_sb[:], goh_d[:])
            ws_sb = const.tile([P, NLAYERS, KD, D], F8, tag="ws")
            nc.sync.dma_start(ws_sb[:], ws_d[:])
            if not bias_zero:
                bias_sb = const.tile([P, NLAYERS, D], BF, tag="bias")
                nc.sync.dma_start(bias_sb[:], bias_d[:])
                ones_sb = const.tile([P, P], BF, tag="ones")
                nc.sync.dma_start(ones_sb[:], ones_d[:])
            wlin_sb = const.tile([P, KD, OUT], BF, tag="wlin")
            nc.sync.dma_start(wlin_sb[:], wlin_d[:])
            blin_sb = const.tile([OUT, 1], F32, tag="blin")
            nc.sync.dma_start(blin_sb[:], blin_d[:])
            invt_sb = const.tile([P, KD * G], F32, tag="invt")
            nc.sync.dma_start(invt_sb[:], invt_d[:])

            def oh_pair(b, t):
                return oh_sb[:].rearrange("p (n q) -> p n q", q=P)[
                    :, b * T_pad + t:b * T_pad + t + 2, :]

            def emit_p_block(xt_src, layer, m, pbuf):
                """p[l=layer] block m = x_l[block m] @ (Wr_hi + 2^-7 Wr_lo),
                into p_shard[pbuf] (fp8, scaled by 2^-t_sh via host weights).
                Column halves so hi+lo PSUM pack into one 2KB bank per buf."""
                H = D // 2
                p_sb = spool.tile([P, D], F8, tag="psb", name=f"psb_{layer}_{m}")
                for h in range(2):
                    # one bank per half-pass: hi accumulates in cols 0:H,
                    # lo in cols H:2H, a single PSUM accumulation group
                    pp = psP.tile([P, D], F32, tag="pp",
                                  name=f"pp_{layer}_{m}_{h}")
                    cs = slice(h * H, (h + 1) * H)
                    for kk in range(KD // 2):
                        nc.tensor.matmul(
                            pp[:, 0:H],
                            lhsT=xt_src[:, 2 * kk:2 * kk + 2, m * P:(m + 1) * P],
                            rhs=wrh_sb[:, layer, 2 * kk:2 * kk + 2, cs],
                            start=(kk == 0), stop=False,
                            perf_mode=DR, skip_group_check=True)
                        nc.tensor.matmul(
                            pp[:, H:D],
                            lhsT=xt_src[:, 2 * kk:2 * kk + 2, m * P:(m + 1) * P],
                            rhs=wrl_sb[:, layer, 2 * kk:2 * kk + 2, cs],
                            start=False, stop=(kk == KD // 2 - 1),
                            perf_mode=DR, skip_group_check=True)
                    # combine: p = hi + 2^-7 * lo   (ACT scales lo, DVE adds)
                    lo_sb = spool.tile([P, H], F32, tag="losb",
                                       name=f"losb_{layer}_{m}_{h}")
                    nc.scalar.activation(
                        lo_sb[:], pp[:, H:D],
                        func=mybir.ActivationFunctionType.Copy,
                        scale=float(2.0 ** -LO_SHIFT))
                    nc.vector.tensor_tensor(p_sb[:, cs], pp[:, 0:H], lo_sb[:],
                                            op=ADD)
                nc.sync.dma_start(
                    p_shard[pbuf][m * P:(m + 1) * P, :], p_sb[:])

            def emit_ag(pbuf):
                if no_cc:
                    nc.sync.dma_start(
                        p_full[pbuf][:NC_NODES, :], p_shard[pbuf][:])
                else:
                    nc.gpsimd.collective_compute(
                        "AllGather", mybir.AluOpType.bypass, replica_groups=RG,
                        ins=[p_shard[pbuf][:]], outs=[p_full[pbuf][:]])

            # prologue: projection for layer 0
            for m in range(NBLK):
                emit_p_block(xt_cur, 0, m, 0)
            emit_ag(0)

            pool_ps = psS.tile([P, KD * G], F32, tag="pool", name="pool_ps")
            for l in range(n_layers):
                pbuf = l % 2
                xs_next = xpool.tile([P, NBLK, D], BF, tag="xs")
                last = l == NLAYERS - 1
                if not last:
                    xt_next = xtpool.tile([P, KD, NC_NODES], F8, tag="xt")
                for b in range(NBLK):
                    g = gpool.tile([P, T_pad, D], F8, tag="g")
                    if no_gather:
                        nc.vector.memset(g[:], 0)
                    else:
                        nsp = min(gsplit, T_pad)
                        th = (T_pad + nsp - 1) // nsp
                        col0 = b * (L // 16)
                        for s0 in range(0, T_pad, th):
                            s1 = min(s0 + th, T_pad)
                            nc.gpsimd.dma_gather(
                                g[:, s0:s1, :], p_full[pbuf][:],
                                idxe_sb[:, col0 + s0 * 8:col0 + s1 * 8],
                                (s1 - s0) * P, (s1 - s0) * P, D,
                                single_packet=False,
                                queue_num=b % nqueues)
                    aps = psA.tile([P, D], F32, tag="aps")
                    # Ws root first: it only needs resident data, so PE
                    # progresses on this block while its gather drains
                    for kk in range(KD // 2):
                        nc.tensor.matmul(
                            aps[:],
                            lhsT=xt_cur[:, 2 * kk:2 * kk + 2, b * P:(b + 1) * P],
                            rhs=ws_sb[:, l, 2 * kk:2 * kk + 2, :],
                            start=(kk == 0), stop=False,
                            perf_mode=DR)
                    if not bias_zero:
                        nc.tensor.matmul(
                            aps[:], lhsT=ones_sb[:], rhs=bias_sb[:, l, :],
                            start=False, stop=False)
                    for tp in range(NPAIR):
                        nc.tensor.matmul(
                            aps[:],
                            lhsT=oh_pair(b, 2 * tp),
                            rhs=g[:, 2 * tp:2 * tp + 2, :],
                            start=False,
                            stop=(TAIL == 0 and tp == NPAIR - 1),
                            perf_mode=DR)
                    if TAIL:
                        nc.tensor.matmul(
                            aps[:],
                            lhsT=oh_sb[:, (b * T_pad + T_pad - 1) * P:
                                       (b * T_pad + T_pad) * P],
                            rhs=g[:, T_pad - 1, :],
                            start=False, stop=True)
                    if last:
                        nc.vector.tensor_tensor(
                            xs_next[:, b, :], aps[:], xs_cur[:, b, :], op=ADD)
                        # pooling partials for this block, interleaved so they
                        # hide under later blocks' gathers
                        for j in range(KD):
                            nc.tensor.matmul(
                                pool_ps[:, j * G:(j + 1) * G],
                                lhsT=xs_next[:, b, j * P:(j + 1) * P],
                                rhs=goh_sb[:, b * G:(b + 1) * G],
                                start=(b == 0 and j == 0),
                                stop=(b == NBLK - 1 and j == KD - 1),
                                skip_group_check=True)
                    else:
                        t1 = spool.tile([P, D], BF, tag="t1")
                        nc.vector.tensor_tensor(
                            t1[:], aps[:], xs_cur[:, b, :], op=ADD)
                        nc.scalar.activation(
                            xs_next[:, b, :], t1[:],
                            func=mybir.ActivationFunctionType.Relu)
                        # transpose new block into xt_next (channel-major,
                        # fp8 at the next layer's x-scale): all 4 chunks into
                        # one PSUM bank as a single group, then ONE DVE copy
                        sc_next = float(2.0 ** -S[l + 1])
                        trp = psT.tile([P, KD * P], BF, tag="tr")
                        for j in range(KD):
                            nc.tensor.matmul(
                                trp[:, j * P:(j + 1) * P],
                                lhsT=xs_next[:, b, j * P:(j + 1) * P],
                                rhs=ident_sb[:],
                                is_transpose=True,
                                start=(j == 0), stop=(j == KD - 1),
                                skip_group_check=True)
                        trv = trp[:].rearrange("p (j q) -> p j q", q=P)
                        if S[l + 1] == 0:
                            nc.vector.tensor_copy(
                                xt_next[:, :, b * P:(b + 1) * P], trv)
                        else:
                            nc.vector.tensor_scalar_mul(
                                xt_next[:, :, b * P:(b + 1) * P], trv, sc_next)
                        # pipelined projection for layer l+1, block b
                        emit_p_block(xt_next, l + 1, b, 1 - pbuf)
                if not last:
                    emit_ag(1 - pbuf)
                    xt_cur = xt_next
                xs_cur = xs_next

            # ---- pooling partials were accumulated inside the last layer's
            # block loop (one PSUM region per 128-channel chunk)
            pool_sb = spool.tile([P, KD * G], F32, tag="pool_sb")
            nc.vector.tensor_copy(pool_sb[:], pool_ps[:])
            nc.sync.dma_start(pool_in[:], pool_sb[:])
            if no_cc:
                nc.sync.dma_start(pool_out[:], pool_sb[:])
            else:
                nc.gpsimd.collective_compute(
                    "AllReduce", ADD, replica_groups=RG,
                    ins=[pool_in[:]], outs=[pool_out[:]])
            pool2 = spool.tile([P, KD * G], F32, tag="pool2")
            nc.sync.dma_start(pool2[:], pool_out[:])
            poolbf = spool.tile([P, KD * G], BF, tag="poolbf")
            nc.vector.tensor_tensor(poolbf[:], pool2[:], invt_sb[:], op=MUL)
            fin_ps = psS.tile([P, G], F32, tag="fin", name="fin_ps")
            for k in range(KD):
                nc.tensor.matmul(
                    fin_ps[:], lhsT=wlin_sb[:, k, :],
                    rhs=poolbf[:, k * G:(k + 1) * G],
                    start=(k == 0), stop=(k == KD - 1))
            fin_sb = spool.tile([OUT, G], F32, tag="fin_sb")
            nc.vector.tensor_tensor(
                fin_sb[:], fin_ps[:], blin_sb[:, :1].to_broadcast([OUT, G]),
                op=ADD)
            nc.sync.dma_start(out_d[:], fin_sb[:])

    nc.compile()
    return nc


def kernel(**inputs):
    import os
    from concourse.bass_utils import run_bass_kernel_spmd

    per_core, shared, meta = _prep(inputs)
    nc = _build(meta)
    in_maps = [{**pc, **shared} for pc in per_core]
    trace = bool(int(os.environ.get("GCN_TRACE", "0")))
    res = run_bass_kernel_spmd(nc, in_maps, core_ids=list(range(NCORES)),
                               trace=trace)
    if trace:
        print(f"HW exec time: {res.exec_time_ns} ns")
        if res.instructions_and_trace is not None:
            print("trace:", res.instructions_and_trace[1])
    out_t = res.results[0]["out_t"]
    return np.ascontiguousarray(out_t.T).astype(np.float32)


# revision 14
# speedup vs baseline: 1.0514x; 1.0070x over previous
"""Distributed GCN (5x GraphConv(add) + residual/ReLU + mean-pool + linear)
for 8 Trainium2 NeuronCores.

Sharding: nodes partitioned contiguously across cores (1280 nodes/core, padded
to 10240). Each core owns the edges whose *destination* lands in its shard.
Aggregation is computed as A@(x@Wr): project first (p = x@Wr), AllGather p,
gather p[src[e]] rows with SWDGE dma_gather, then reduce edge tiles onto
destination nodes with one-hot segment matmuls on the tensor engine.

fp8 pipeline: p is stored/gathered as fp8e4m3 (halves gather DMA bytes), and
all heavy matmuls run in fp8 DoubleRow perf mode (2 K-slabs per instruction,
0.5 cycles/row): segment one-hot pairs, Ws root pairs, and Wr projection as an
fp8 hi + 2^7-scaled-lo pair (the lo PSUM is recombined at 2^-7 so the
effective Wr precision exceeds bf16). Per-layer power-of-2 x-scales (computed
from a host forward pass) keep activations inside fp8e4m3 range; they are
folded into the host-prepped weights so no extra device ops are needed except
a scaled copy when building the transposed x. The residual stream, pooling,
and final linear stay bf16/fp32.
"""

import numpy as np
import ml_dtypes

BF16 = ml_dtypes.bfloat16
FP8 = ml_dtypes.float8_e4m3      # TRN float8e4 (max normal 240)

N, E, D, OUT, G = 10000, 160000, 512, 128, 64
NCORES, P = 8, 128
NBLK = 10                     # 128-node blocks per core
NC_NODES = NBLK * P           # 1280
NPAD = NCORES * NC_NODES      # 10240
NLAYERS = 5
KD = D // P                   # 4 chunks of in-channels
LO_SHIFT = 7                  # Wr lo-residual stored scaled by 2^LO_SHIFT


def _wrap_idx(a):
    """[L] ints -> [128, L//16] int16 SWDGE index layout (16-partition wrap,
    replicated for the 8 Q7 cores)."""
    L = len(a)
    w = a.astype(np.int16).reshape(L // 16, 16).T
    return np.ascontiguousarray(np.tile(w, (8, 1)))


def _forward_stats(x, src, dst, batch, inputs):
    """fp32 host forward pass -> per-layer max|x_in| and max|p|."""
    try:
        import scipy.sparse as sp
        A = sp.csr_matrix((np.ones(len(src), np.float32), (dst, src)), shape=(N, N))
        segsum = lambda v: A @ v
    except Exception:
        def segsum(v):
            out = np.zeros((N, v.shape[1]), np.float32)
            np.add.at(out, dst, v[src])
            return out
    xmax, pmax = [], []
    xx = x
    for l in range(NLAYERS):
        Wr = np.asarray(inputs[f"Wr{l+1}"], np.float32)
        Ws = np.asarray(inputs[f"Ws{l+1}"], np.float32)
        b = np.asarray(inputs[f"b{l+1}"], np.float32)
        xmax.append(float(np.abs(xx).max()))
        p = xx @ Wr
        pmax.append(float(np.abs(p).max()))
        val = segsum(p) + xx @ Ws + b + xx
        if l < NLAYERS - 1:
            val = np.maximum(val, 0)
        xx = val
    return xmax, pmax


def _prep(inputs):
    x = np.asarray(inputs["x"], np.float32)
    ei = np.asarray(inputs["edge_index"]).astype(np.int64)
    batch = np.asarray(inputs["batch"]).astype(np.int64)
    src, dst = ei[0], ei[1]

    xmax, pmax = _forward_stats(x, src, dst, batch, inputs)
    # x scale: keep x * 2^-S[l] <= ~200 (fp8e4m3 max normal 240)
    S = [max(0, int(np.ceil(np.log2(m / 200.0)))) if m > 200 else 0 for m in xmax]
    # p scale: keep p * 2^-t <= ~200 (one-hot value 2^t restores it)
    pm = max(pmax[l] * 2.0 ** 0 for l in range(NLAYERS))
    t_sh = max(0, int(np.ceil(np.log2(pm / 200.0)))) if pm > 200 else 0

    order = np.argsort(dst, kind="stable")
    ds_, ss_ = dst[order], src[order]
    starts = np.searchsorted(ds_, np.arange(0, NPAD + 1, P))
    counts = np.diff(starts)
    T_pad = max(1, int(np.ceil(counts.max() / P)))
    L = T_pad * P

    xp = np.zeros((NPAD, D), np.float32)
    xp[:N] = x

    counts_g = np.bincount(batch, minlength=G)[:G]
    inv = (1.0 / np.maximum(counts_g, 1.0)).astype(np.float32)

    oh_val = float(2.0 ** t_sh)
    per_core = []
    for c in range(NCORES):
        idx_blocks = []
        oh_flat = np.zeros((P, NBLK * T_pad * P), FP8)
        goh = np.zeros((P, NBLK * G), BF16)
        for b in range(NBLK):
            gb = c * NBLK + b
            lo = gb * P
            s0, s1 = int(starts[gb]), int(starts[gb + 1])
            n = s1 - s0
            srcs = np.zeros(L, np.int64)
            srcs[:n] = ss_[s0:s1]
            dloc = ds_[s0:s1] - lo
            oh = np.zeros((L, P), FP8)
            oh[np.arange(n), dloc] = oh_val
            idx_blocks.append(_wrap_idx(srcs))
            oh_flat[:, b * T_pad * P:(b + 1) * T_pad * P] = (
                oh.reshape(T_pad, P, P).transpose(1, 0, 2).reshape(P, T_pad * P))
            nodes = lo + np.arange(P)
            valid = nodes < N
            goh[valid, b * G + batch[nodes[valid]]] = 1

        shard = xp[c * NC_NODES:(c + 1) * NC_NODES].astype(BF16)
        xt0 = np.ascontiguousarray(
            (shard.astype(np.float32) * 2.0 ** -S[0]).astype(FP8)
            .T.reshape(KD, P, NC_NODES).transpose(1, 0, 2))
        per_core.append(dict(
            x_shard=np.ascontiguousarray(shard),
            xt0=xt0,
            ohot=oh_flat,
            idxe=np.ascontiguousarray(np.concatenate(idx_blocks, axis=1)),
            goh=goh,
        ))

    wr_hi = np.zeros((P, NLAYERS, KD, D), FP8)
    wr_lo = np.zeros((P, NLAYERS, KD, D), FP8)
    ws_q = np.zeros((P, NLAYERS, KD, D), FP8)
    bias = np.zeros((P, NLAYERS, D), BF16)
    bias_zero = True
    for l in range(NLAYERS):
        WR = np.asarray(inputs[f"Wr{l+1}"], np.float32) * 2.0 ** (S[l] - t_sh)
        hi = WR.astype(FP8)
        lo = ((WR - hi.astype(np.float32)) * 2.0 ** LO_SHIFT).astype(FP8)
        WS = (np.asarray(inputs[f"Ws{l+1}"], np.float32) * 2.0 ** S[l]).astype(FP8)
        for arr, dst_arr in ((hi, wr_hi), (lo, wr_lo), (WS, ws_q)):
            dst_arr[:, l] = arr.reshape(KD, P, D).transpose(1, 0, 2)
        b_l = np.asarray(inputs[f"b{l+1}"], np.float32)
        if np.any(b_l != 0):
            bias_zero = False
        bias[0, l] = b_l.astype(BF16)
    ones_e0 = np.zeros((P, P), BF16)
    ones_e0[0, :] = 1
    wlin = np.ascontiguousarray(
        np.asarray(inputs["Wlin"], np.float32).reshape(KD, P, OUT)
        .transpose(1, 0, 2).astype(BF16))
    blin = np.asarray(inputs["blin"], np.float32).reshape(OUT, 1).astype(np.float32)
    shared = dict(
        wr_hi=wr_hi, wr_lo=wr_lo, ws=ws_q, bias=bias, ones=ones_e0,
        wlin=wlin, blin=blin,
        invt=np.ascontiguousarray(np.tile(inv, (P, KD)).astype(np.float32)),
        ident=np.eye(P, dtype=BF16),
    )
    meta = dict(T_pad=T_pad, S=S, t_sh=t_sh, bias_zero=bias_zero)
    return per_core, shared, meta


def _unwrap(w, L):
    """inverse of _wrap_idx: [128, L//16] -> [L]"""
    return np.ascontiguousarray(w[:16].T).reshape(-1)[:L].astype(np.int64)


def emulate(inputs):
    """Numpy emulation of the exact device dataflow (fp8/bf16 casts included).
    Validates the host-side scale/index/one-hot bookkeeping."""
    per_core, shared, meta = _prep(inputs)
    T_pad, S, t_sh = meta["T_pad"], meta["S"], meta["t_sh"]
    L = T_pad * P
    f32 = np.float32

    xs = [pc["x_shard"].astype(f32) for pc in per_core]       # [1280, 512]
    xts = [pc["xt0"] for pc in per_core]                      # fp8 [P, KD, NC]
    for l in range(NLAYERS):
        wr_hi = np.concatenate([shared["wr_hi"][:, l, k, :] for k in range(KD)],
                               axis=0).astype(f32)
        wr_lo = np.concatenate([shared["wr_lo"][:, l, k, :] for k in range(KD)],
                               axis=0).astype(f32)
        ws_l = np.concatenate([shared["ws"][:, l, k, :] for k in range(KD)],
                              axis=0).astype(f32)
        b_l = shared["bias"][0, l].astype(f32)
        p_parts = []
        for c in range(NCORES):
            xm = np.concatenate(
                [xts[c][:, k, :].astype(f32) for k in range(KD)], axis=0).T
            p = xm @ wr_hi + (xm @ wr_lo) * 2.0 ** -LO_SHIFT
            p_parts.append(p.astype(FP8).astype(f32))
        p_full = np.concatenate(p_parts, axis=0)              # [10240, 512]
        new_xs, new_xts = [], []
        for c in range(NCORES):
            nx = np.zeros((NC_NODES, D), f32)
            nxt = np.zeros((P, KD, NC_NODES), FP8)
            xm_all = np.concatenate(
                [xts[c][:, k, :].astype(f32) for k in range(KD)], axis=0).T
            for b in range(NBLK):
                idx = _unwrap(
                    per_core[c]["idxe"][:, b * (L // 16):(b + 1) * (L // 16)], L)
                gath = p_full[idx]                             # [L, 512]
                acc = np.zeros((P, D), f32)
                for t in range(T_pad):
                    oh = per_core[c]["ohot"][
                        :, (b * T_pad + t) * P:(b * T_pad + t + 1) * P
                    ].astype(f32)                             # [128e, 128d]
                    acc += oh.T @ gath[t * P:(t + 1) * P]
                acc += xm_all[b * P:(b + 1) * P] @ ws_l + b_l
                val = acc + xs[c][b * P:(b + 1) * P]
                if l < NLAYERS - 1:
                    val = np.maximum(val, 0)
                val = val.astype(BF16).astype(f32)
                nx[b * P:(b + 1) * P] = val
                if l < NLAYERS - 1:
                    sc = 2.0 ** -S[l + 1]
                    nxt[:, :, b * P:(b + 1) * P] = (
                        (val * sc).astype(FP8).T.reshape(KD, P, P)
                        .transpose(1, 0, 2))
            new_xs.append(nx)
            new_xts.append(nxt)
        xs, xts = new_xs, new_xts
    # pooling
    pooled_T = np.zeros((D, G), f32)
    for c in range(NCORES):
        goh = per_core[c]["goh"].astype(f32)
        for b in range(NBLK):
            blk = xs[c][b * P:(b + 1) * P].astype(BF16).astype(f32)
            for j in range(KD):
                pooled_T[j * P:(j + 1) * P] += (
                    blk[:, j * P:(j + 1) * P].T @ goh[:, b * G:(b + 1) * G])
    inv = shared["invt"][0, :G].astype(f32)
    pooled_T = (pooled_T * inv[None, :]).astype(BF16).astype(f32)
    wlin = np.concatenate([shared["wlin"][:, k, :] for k in range(KD)],
                          axis=0).astype(f32)                 # [512, 128]
    out_T = wlin.T @ pooled_T + shared["blin"][:, :1]         # [128, 64]
    return np.ascontiguousarray(out_T.T).astype(np.float32)


def _build(meta, enable_asserts=False):
    import os
    T_pad = meta["T_pad"]
    S, bias_zero = meta["S"], meta["bias_zero"]
    n_layers = int(os.environ.get("GCN_LAYERS", NLAYERS))
    no_gather = bool(int(os.environ.get("GCN_NO_GATHER", "0")))
    no_cc = bool(int(os.environ.get("GCN_NO_CC", "0")))
    bP, bA, bT = (int(v) for v in os.environ.get("GCN_BANKS", "3,2,1").split(","))
    gbufs = int(os.environ.get("GCN_GBUFS", "4"))
    gsplit = int(os.environ.get("GCN_GSPLIT", "2"))
    # SWDGE ring: big enough for 2 block-gathers so descriptor generation for
    # block b+1 overlaps block b's DMA drain; 2 queues decouple them further
    scratch = int(os.environ.get("GCN_SCRATCH", "16384"))
    nqueues = int(os.environ.get("GCN_NQUEUES", "2"))
    import concourse.bass as bass
    import concourse.mybir as mybir
    import concourse.tile as tile
    from concourse import bacc

    F32 = mybir.dt.float32
    BF = mybir.dt.bfloat16
    F8 = mybir.dt.float8e4
    I16 = mybir.dt.int16
    ADD = mybir.AluOpType.add
    MUL = mybir.AluOpType.mult
    DR = mybir.MatmulPerfMode.DoubleRow
    L = T_pad * P
    RG = [list(range(NCORES))]
    NPAIR, TAIL = T_pad // 2, T_pad % 2

    nc = bacc.Bacc("TRN2", target_bir_lowering=False, debug=False,
                   enable_asserts=enable_asserts, num_devices=NCORES,
                   dynamic_dma_scratch_size=scratch,
                   num_swdge_queues=nqueues)

    # per-core inputs
    x_d = nc.dram_tensor("x_shard", [NC_NODES, D], BF, kind="ExternalInput")
    xt0_d = nc.dram_tensor("xt0", [P, KD, NC_NODES], F8, kind="ExternalInput")
    oh_d = nc.dram_tensor("ohot", [P, NBLK * T_pad * P], F8, kind="ExternalInput")
    idxe_d = nc.dram_tensor("idxe", [P, NBLK * (L // 16)], I16, kind="ExternalInput")
    goh_d = nc.dram_tensor("goh", [P, NBLK * G], BF, kind="ExternalInput")
    # shared inputs
    wrh_d = nc.dram_tensor("wr_hi", [P, NLAYERS, KD, D], F8, kind="ExternalInput")
    wrl_d = nc.dram_tensor("wr_lo", [P, NLAYERS, KD, D], F8, kind="ExternalInput")
    ws_d = nc.dram_tensor("ws", [P, NLAYERS, KD, D], F8, kind="ExternalInput")
    bias_d = nc.dram_tensor("bias", [P, NLAYERS, D], BF, kind="ExternalInput")
    ones_d = nc.dram_tensor("ones", [P, P], BF, kind="ExternalInput")
    wlin_d = nc.dram_tensor("wlin", [P, KD, OUT], BF, kind="ExternalInput")
    blin_d = nc.dram_tensor("blin", [OUT, 1], F32, kind="ExternalInput")
    invt_d = nc.dram_tensor("invt", [P, KD * G], F32, kind="ExternalInput")
    ident_d = nc.dram_tensor("ident", [P, P], BF, kind="ExternalInput")
    # internal DRAM (double-buffered by layer parity so the AllGather for
    # layer l+1 never WAR-depends on layer l's gathers)
    p_shard = [nc.dram_tensor(f"p_shard{i}", [NC_NODES, D], F8) for i in (0, 1)]
    p_full = [nc.dram_tensor(f"p_full{i}", [NPAD, D], F8, addr_space="Shared")
              for i in (0, 1)]
    pool_in = nc.dram_tensor("pool_in", [P, KD * G], F32)
    pool_out = nc.dram_tensor("pool_out", [P, KD * G], F32, addr_space="Shared")
    # output
    out_d = nc.dram_tensor("out_t", [OUT, G], F32, kind="ExternalOutput")

    with tile.TileContext(nc) as tc:
        with (
            tc.tile_pool(name="const", bufs=1) as const,
            tc.tile_pool(name="xs", bufs=2) as xpool,
            tc.tile_pool(name="xt", bufs=2) as xtpool,
            tc.tile_pool(name="gath", bufs=gbufs) as gpool,
            tc.tile_pool(name="small", bufs=int(os.environ.get("GCN_SBUFS", "4"))) as spool,
            tc.tile_pool(name="psP", bufs=bP, space="PSUM") as psP,
            tc.tile_pool(name="psA", bufs=bA, space="PSUM") as psA,
            tc.tile_pool(name="psS", bufs=1, space="PSUM") as psS,
            tc.tile_pool(name="psT", bufs=bT, space="PSUM") as psT,
        ):
            # ---- constants to SBUF (prologue-projection inputs first so
            # the PE starts while the big one-hot/index tables stream in)
            wrh_sb = const.tile([P, NLAYERS, KD, D], F8, tag="wrh")
            nc.sync.dma_start(wrh_sb[:], wrh_d[:])
            wrl_sb = const.tile([P, NLAYERS, KD, D], F8, tag="wrl")
            nc.sync.dma_start(wrl_sb[:], wrl_d[:])
            ident_sb = const.tile([P, P], BF, tag="ident")
            nc.sync.dma_start(ident_sb[:], ident_d[:])
            xs_cur = xpool.tile([P, NBLK, D], BF, tag="xs")
            nc.sync.dma_start(xs_cur[:], x_d.ap().rearrange("(b p) d -> p b d", p=P))
            xt_cur = xtpool.tile([P, KD, NC_NODES], F8, tag="xt")
            nc.sync.dma_start(xt_cur[:], xt0_d[:])
            oh_sb = const.tile([P, NBLK * T_pad * P], F8, tag="oh")
            nc.sync.dma_start(oh_sb[:], oh_d[:])
            idxe_sb = const.tile([P, NBLK * (L // 16)], I16, tag="idxe")
            nc.sync.dma_start(idxe_sb[:], idxe_d[:])
            goh_sb = const.tile([P, NBLK * G], BF, tag="goh")
            nc.sync.dma_start(goh_sb[:], goh_d[:])
            ws_sb = const.tile([P, NLAYERS, KD, D], F8, tag="ws")
            nc.sync.dma_start(ws_sb[:], ws_d[:])
            if not bias_zero:
                bias_sb = const.tile([P, NLAYERS, D], BF, tag="bias")
                nc.sync.dma_start(bias_sb[:], bias_d[:])
                ones_sb = const.tile([P, P], BF, tag="ones")
                nc.sync.dma_start(ones_sb[:], ones_d[:])
            wlin_sb = const.tile([P, KD, OUT], BF, tag="wlin")
            nc.sync.dma_start(wlin_sb[:], wlin_d[:])
            blin_sb = const.tile([OUT, 1], F32, tag="blin")
            nc.sync.dma_start(blin_sb[:], blin_d[:])
            invt_sb = const.tile([P, KD * G], F32, tag="invt")
            nc.sync.dma_start(invt_sb[:], invt_d[:])

            def oh_pair(b, t):
                return oh_sb[:].rearrange("p (n q) -> p n q", q=P)[
                    :, b * T_pad + t:b * T_pad + t + 2, :]

            def emit_p_block(xt_src, layer, m, pbuf):
                """p[l=layer] block m = x_l[block m] @ (Wr_hi + 2^-7 Wr_lo),
                into p_shard[pbuf] (fp8, scaled by 2^-t_sh via host weights).
                Column halves so hi+lo PSUM pack into one 2KB bank per buf."""
                H = D // 2
                p_sb = spool.tile([P, D], F8, tag="psb", name=f"psb_{layer}_{m}")
                for h in range(2):
                    # one bank per half-pass: hi accumulates in cols 0:H,
                    # lo in cols H:2H, a single PSUM accumulation group
                    pp = psP.tile([P, D], F32, tag="pp",
                                  name=f"pp_{layer}_{m}_{h}")
                    cs = slice(h * H, (h + 1) * H)
                    for kk in range(KD // 2):
                        nc.tensor.matmul(
                            pp[:, 0:H],
                            lhsT=xt_src[:, 2 * kk:2 * kk + 2, m * P:(m + 1) * P],
                            rhs=wrh_sb[:, layer, 2 * kk:2 * kk + 2, cs],
                            start=(kk == 0), stop=False,
                            perf_mode=DR, skip_group_check=True)
                        nc.tensor.matmul(
                            pp[:, H:D],
                            lhsT=xt_src[:, 2 * kk:2 * kk + 2, m * P:(m + 1) * P],
                            rhs=wrl_sb[:, layer, 2 * kk:2 * kk + 2, cs],
                            start=False, stop=(kk == KD // 2 - 1),
                            perf_mode=DR, skip_group_check=True)
                    # combine: p = hi + 2^-7 * lo   (ACT scales lo, DVE adds)
                    lo_sb = spool.tile([P, H], F32, tag="losb",
                                       name=f"losb_{layer}_{m}_{h}")
                    nc.scalar.activation(
                        lo_sb[:], pp[:, H:D],
                        func=mybir.ActivationFunctionType.Copy,
                        scale=float(2.0 ** -LO_SHIFT))
                    nc.vector.tensor_tensor(p_sb[:, cs], pp[:, 0:H], lo_sb[:],
                                            op=ADD)
                nc.sync.dma_start(
                    p_shard[pbuf][m * P:(m + 1) * P, :], p_sb[:])

            def emit_ag(pbuf):
                if no_cc:
                    nc.sync.dma_start(
                        p_full[pbuf][:NC_NODES, :], p_shard[pbuf][:])
                else:
                    nc.gpsimd.collective_compute(
                        "AllGather", mybir.AluOpType.bypass, replica_groups=RG,
                        ins=[p_shard[pbuf][:]], outs=[p_full[pbuf][:]])

            # prologue: projection for layer 0
            for m in range(NBLK):
                emit_p_block(xt_cur, 0, m, 0)
            emit_ag(0)

            pool_ps = psS.tile([P, KD * G], F32, tag="pool", name="pool_ps")
            for l in range(n_layers):
                pbuf = l % 2
                xs_next = xpool.tile([P, NBLK, D], BF, tag="xs")
                last = l == NLAYERS - 1
                if not last:
                    xt_next = xtpool.tile([P, KD, NC_NODES], F8, tag="xt")
                for b in range(NBLK):
                    g = gpool.tile([P, T_pad, D], F8, tag="g")
                    if no_gather:
                        nc.vector.memset(g[:], 0)
                    else:
                        nsp = min(gsplit, T_pad)
                        th = (T_pad + nsp - 1) // nsp
                        col0 = b * (L // 16)
                        for s0 in range(0, T_pad, th):
                            s1 = min(s0 + th, T_pad)
                            nc.gpsimd.dma_gather(
                                g[:, s0:s1, :], p_full[pbuf][:],
                                idxe_sb[:, col0 + s0 * 8:col0 + s1 * 8],
                                (s1 - s0) * P, (s1 - s0) * P, D,
                                single_packet=False,
                                queue_num=b % nqueues)
                    aps = psA.tile([P, D], F32, tag="aps")
                    # Ws root first: it only needs resident data, so PE
                    # progresses on this block while its gather drains
                    for kk in range(KD // 2):
                        nc.tensor.matmul(
                            aps[:],
                            lhsT=xt_cur[:, 2 * kk:2 * kk + 2, b * P:(b + 1) * P],
                            rhs=ws_sb[:, l, 2 * kk:2 * kk + 2, :],
                            start=(kk == 0), stop=False,
                            perf_mode=DR)
                    if not bias_zero:
                        nc.tensor.matmul(
                            aps[:], lhsT=ones_sb[:], rhs=bias_sb[:, l, :],
                            start=False, stop=False)
                    for tp in range(NPAIR):
                        nc.tensor.matmul(
                            aps[:],
                            lhsT=oh_pair(b, 2 * tp),
                            rhs=g[:, 2 * tp:2 * tp + 2, :],
                            start=False,
                            stop=(TAIL == 0 and tp == NPAIR - 1),
                            perf_mode=DR)
                    if TAIL:
                        nc.tensor.matmul(
                            aps[:],
                            lhsT=oh_sb[:, (b * T_pad + T_pad - 1) * P:
                                       (b * T_pad + T_pad) * P],
                            rhs=g[:, T_pad - 1, :],
                            start=False, stop=True)
                    if last:
                        nc.vector.tensor_tensor(
                            xs_next[:, b, :], aps[:], xs_cur[:, b, :], op=ADD)
                        # pooling partials for this block, interleaved so they
                        # hide under later blocks' gathers
                        for j in range(KD):
                            nc.tensor.matmul(
                                pool_ps[:, j * G:(j + 1) * G],
                                lhsT=xs_next[:, b, j * P:(j + 1) * P],
                                rhs=goh_sb[:, b * G:(b + 1) * G],
                                start=(b == 0 and j == 0),
                                stop=(b == NBLK - 1 and j == KD - 1),
                                skip_group_check=True)
                    else:
                        t1 = spool.tile([P, D], BF, tag="t1")
                        nc.vector.tensor_tensor(
                            t1[:], aps[:], xs_cur[:, b, :], op=ADD)
                        nc.scalar.activation(
                            xs_next[:, b, :], t1[:],
                            func=mybir.ActivationFunctionType.Relu)
                        # transpose new block into xt_next (channel-major,
                        # fp8 at the next layer's x-scale): all 4 chunks into
                        # one PSUM bank as a single group, then ONE DVE copy
                        sc_next = float(2.0 ** -S[l + 1])
                        trp = psT.tile([P, KD * P], BF, tag="tr")
                        for j in range(KD):
                            nc.tensor.matmul(
                                trp[:, j * P:(j + 1) * P],
                                lhsT=xs_next[:, b, j * P:(j + 1) * P],
                                rhs=ident_sb[:],
                                is_transpose=True,
                                start=(j == 0), stop=(j == KD - 1),
                                skip_group_check=True)
                        trv = trp[:].rearrange("p (j q) -> p j q", q=P)
                        if S[l + 1] == 0:
                            nc.vector.tensor_copy(
                                xt_next[:, :, b * P:(b + 1) * P], trv)
                        else:
                            nc.vector.tensor_scalar_mul(
                                xt_next[:, :, b * P:(b + 1) * P], trv, sc_next)
                        # pipelined projection for layer l+1, block b
                        emit_p_block(xt_next, l + 1, b, 1 - pbuf)
                if not last:
                    emit_ag(1 - pbuf)
                    xt_cur = xt_next
                xs_cur = xs_next

            # ---- pooling partials were accumulated inside the last layer's
            # block loop (one PSUM region per 128-channel chunk)
            pool_sb = spool.tile([P, KD * G], F32, tag="pool_sb")
            nc.vector.tensor_copy(pool_sb[:], pool_ps[:])
            nc.sync.dma_start(pool_in[:], pool_sb[:])
            if no_cc:
                nc.sync.dma_start(pool_out[:], pool_sb[:])
            else:
                nc.gpsimd.collective_compute(
                    "AllReduce", ADD, replica_groups=RG,
                    ins=[pool_in[:]], outs=[pool_out[:]])
            pool2 = spool.tile([P, KD * G], F32, tag="pool2")
            nc.sync.dma_start(pool2[:], pool_out[:])
            poolbf = spool.tile([P, KD * G], BF, tag="poolbf")
            nc.vector.tensor_tensor(poolbf[:], pool2[:], invt_sb[:], op=MUL)
            fin_ps = psS.tile([P, G], F32, tag="fin", name="fin_ps")
            for k in range(KD):
                nc.tensor.matmul(
                    fin_ps[:], lhsT=wlin_sb[:, k, :],
                    rhs=poolbf[:, k * G:(k + 1) * G],
                    start=(k == 0), stop=(k == KD - 1))
            fin_sb = spool.tile([OUT, G], F32, tag="fin_sb")
            nc.vector.tensor_tensor(
                fin_sb[:], fin_ps[:], blin_sb[:, :1].to_broadcast([OUT, G]),
                op=ADD)
            nc.sync.dma_start(out_d[:], fin_sb[:])

    nc.compile()
    return nc


def kernel(**inputs):
    import os
    from concourse.bass_utils import run_bass_kernel_spmd

    per_core, shared, meta = _prep(inputs)
    nc = _build(meta)
    in_maps = [{**pc, **shared} for pc in per_core]
    trace = bool(int(os.environ.get("GCN_TRACE", "0")))
    res = run_bass_kernel_spmd(nc, in_maps, core_ids=list(range(NCORES)),
                               trace=trace)
    if trace:
        print(f"HW exec time: {res.exec_time_ns} ns")
        if res.instructions_and_trace is not None:
            print("trace:", res.instructions_and_trace[1])
    out_t = res.results[0]["out_t"]
    return np.ascontiguousarray(out_t.T).astype(np.float32)
